# revision 10
# baseline (speedup 1.0000x reference)
# Trainium2 Bass kernel for nn_AutoRegressive (LSTM warmup + autoregressive decode).
#
# Problem: B=512, T=128, F=64, UNITS=1024, OUT_STEPS=32.
#   warmup: 128 sequential LSTM steps over inputs, keep final (h, c)
#   decode: pred = h @ Wd + bd, feed pred back as x for 31 more steps
#   output: [B, 32, F]
#
# Strategy: pure 8-way data parallelism on the batch axis (64 rows/core),
# weights replicated, zero cross-core communication. Per step the dominant
# matmul z = x @ Wk + h @ Wr is computed with h^T-stationary matmuls
# (lhsT = h^T[k-chunk] [128, 64]) streaming Wr columns. Because the local
# batch is 64 (< 128 array columns), two matmuls are column-tiled at
# tile_position (0,0)/(0,64) to process the lo/hi unit-halves of each gate
# concurrently (emitted adjacently so the PE overlaps them), keeping the
# 128x128 PE array fully utilized.
# All matmul operands are bf16 (PSUM accumulates f32); gates/state are f32.
# h -> h^T via 4 PE transposes per step; each transpose PAIR has its own
# PSUM bank and its own hT SBUF tile (hTa/hTb), and the next step's k-loop
# is ordered by chunk readiness so the pair-0 copy unblocks half of it
# while pair 1 is in flight. The g and o gates are split into 2x256-col
# PSUM tiles so the c/h gate chain pipelines with the matmul tail; zero-
# accumulate filler matmuls absorb the remaining PE wait windows (keeps
# the HAM clock gate at 8/8). Decode runs h@Wr first and x@Wk last so the
# pred -> x_dec chain hides under the matmuls; pred copies run on ScalarE
# with bd folded in as a per-partition Identity bias. Bias b is folded
# into an augmented ones-row of x / extra row of Wk on the host.
# Measured: 1.811 ms exec on hardware, rel err 3.4e-3 vs the reference.
import os
import sys

sys.path.insert(0, "/opt/trn_rl_repo")

import numpy as np
import ml_dtypes

import concourse.bass as bass
import concourse.mybir as mybir
import concourse.tile as tile
from concourse import bacc
from concourse.bass_utils import run_bass_kernel_spmd
from concourse.masks import make_identity
from contextlib import ExitStack

F32, BF16 = mybir.dt.float32, mybir.dt.bfloat16
AF = mybir.ActivationFunctionType
Alu = mybir.AluOpType

B_FULL, T_FULL, F_DIM, UNITS = 512, 128, 64, 1024
N_CORES = 8
B = B_FULL // N_CORES          # 64 local batch rows
NK = UNITS // 128              # 8 k-chunks of the recurrent contraction
GATES = [1, 0, 2, 3]           # processing order f,i,g,o (orig packing i,f,g,o)

_NC_CACHE = {}


def _build(n_warm: int, n_out: int):
    """Build the per-core Bass program. n_out = number of predictions (32)."""
    key = (n_warm, n_out)
    if key in _NC_CACHE:
        return _NC_CACHE[key]

    n_dec = n_out - 1  # LSTM steps in decode phase

    nc = bacc.Bacc("TRN2", target_bir_lowering=False, debug=False,
                   num_devices=N_CORES)
    xt_ext = nc.dram_tensor("xt", [n_warm, F_DIM + 1, B], BF16,
                            kind="ExternalInput")
    wr_ext = nc.dram_tensor("wr", [128, NK * 8 * 512], BF16,
                            kind="ExternalInput")
    wk_ext = nc.dram_tensor("wk", [F_DIM + 1, 8 * 512], BF16,
                            kind="ExternalInput")
    wd_ext = nc.dram_tensor("wd", [128, NK * F_DIM], BF16,
                            kind="ExternalInput")
    bd_ext = nc.dram_tensor("bd", [F_DIM, 1], F32, kind="ExternalInput")
    out_ext = nc.dram_tensor("out", [F_DIM, n_out * B], F32,
                             kind="ExternalOutput")

    with ExitStack() as ctx:
        tc = ctx.enter_context(tile.TileContext(nc))
        wpool = ctx.enter_context(tc.tile_pool(name="w", bufs=1))
        xpool = ctx.enter_context(tc.tile_pool(name="x", bufs=3))
        hTpool = ctx.enter_context(tc.tile_pool(name="hT", bufs=2))
        hpool = ctx.enter_context(tc.tile_pool(name="h", bufs=3))
        cpool = ctx.enter_context(tc.tile_pool(name="c", bufs=3))
        gpool = ctx.enter_context(tc.tile_pool(name="g", bufs=3))
        xdpool = ctx.enter_context(tc.tile_pool(name="xd", bufs=2))
        zpool = ctx.enter_context(tc.tile_pool(name="z", bufs=1, space="PSUM"))
        zspool = ctx.enter_context(tc.tile_pool(name="zs", bufs=2, space="PSUM"))
        tpool = ctx.enter_context(tc.tile_pool(name="tp", bufs=1, space="PSUM"))

        wr_sb = wpool.tile([128, NK * 8 * 512], BF16)
        nc.sync.dma_start(wr_sb[:], wr_ext[:])
        wk_sb = wpool.tile([F_DIM + 1, 8 * 512], BF16)
        nc.sync.dma_start(wk_sb[:], wk_ext[:])
        wd_sb = wpool.tile([128, NK * F_DIM], BF16)
        nc.sync.dma_start(wd_sb[:], wd_ext[:])
        bd_sb = wpool.tile([F_DIM, 1], F32)
        nc.sync.dma_start(bd_sb[:], bd_ext[:])
        identb = wpool.tile([128, 128], BF16)
        make_identity(nc, identb[:])
        ones_sb = wpool.tile([1, B], BF16)
        nc.vector.memset(ones_sb[:], 1.0)
        zeros_sb = wpool.tile([128, 512], BF16)
        nc.vector.memset(zeros_sb[:], 0.0)
        preds_sb = wpool.tile([F_DIM, n_out * B], F32)
        xd_sb = wpool.tile([F_DIM + 1, B], BF16)
        nc.vector.memset(xd_sb[F_DIM:F_DIM + 1, :], 1.0)

        state = {"h0": None, "h1": None, "c": None, "hT": None}
        # hT column layout: transpose of h[:, j*128:(j+1)*128] yields unit
        # chunks j (cols 0:64) and j+4 (cols 64:128); store them adjacently
        # so each transpose pair needs ONE contiguous DVE copy.
        HT_POS = {}
        for j in range(4):
            HT_POS[j] = 2 * j
            HT_POS[j + 4] = 2 * j + 1

        def hT_sl(k):
            p = HT_POS[k]
            t = state["hTa"] if p < 4 else state["hTb"]
            return t[:, (p % 4) * B:(p % 4 + 1) * B]

        def transposes(js):
            """h halves (bf16, batch-major split layout) -> hT chunks (bf16).

            transpose of h[:, j*128:(j+1)*128] yields unit chunks j and j+4
            side by side. Each pair gets its own PSUM bank and its own hT
            SBUF tile so the copy of pair 0 (chunks 0,4,1,5) unblocks the
            first half of the next step's k-loop while pair 1 is still in
            flight."""
            # Each transpose pair gets its OWN PSUM bank and its own hT
            # SBUF tile: the pair-0 copy (chunks 0,4,1,5) unblocks the first
            # half of the next step's k-loop while pair 1 and its copy are
            # still in flight — real work instead of filler in the
            # HAM-activity window.
            half = js[0] // 2
            tag = "tp0" if half == 0 else "tps"
            tps = tpool.tile([128, 1024], BF16, name=tag, tag=tag)[:, 0:256]
            hT = state["hTa"] if half == 0 else state["hTb"]
            for jj, j in enumerate(js):
                h_half = state["h0"] if j < 2 else state["h1"]
                nc.tensor.transpose(tps[:, jj * 128:(jj + 1) * 128],
                                    h_half[:, (j % 2) * 128:(j % 2 + 1) * 128],
                                    identb[:])
            nc.vector.tensor_copy(hT[:], tps[:])

        def keep_warm(zs, n, start=False):
            """Filler matmuls accumulating an all-zeros K=1 product into the
            live zf tile: numerically a no-op, but real PE activity (keeps
            the HAM clock gate at 8/8 across the per-step gate-chain tail)
            that writes a consumed tile (so DCE keeps it). With start=True
            the first one opens zf's group (decode, where x@Wk comes last)."""
            zf = zs[0]
            for i in range(n):
                nc.tensor.matmul(zf[0:64, :], wr_sb[:, 0:64], zeros_sb[:],
                                 start=(start and i == 0), stop=False)

        def pred_block(d):
            """pred_d^T = Wd^T @ h + bd from current hT; returns x_dec tile."""
            # share the pair-0 transpose bank: it is released right after
            # copy-a (early), so the pred matmuls start ~0.7us sooner than
            # waiting for copy-b's slot. Safe now that the pred copies run
            # on ScalarE (the old DVE-queue release stall is gone).
            pp = tpool.tile([F_DIM, 512], F32, name="pp", tag="tp0")[:, 0:B]
            for ki, k in enumerate((0, 4, 1, 5, 2, 6, 3, 7)):
                nc.tensor.matmul(pp[:], wd_sb[:, k * F_DIM:(k + 1) * F_DIM],
                                 hT_sl(k), start=(k == 0), stop=(ki == 7))
            # Copies on ScalarE (idle here, and off the DVE queue which is
            # busy with the hT copies); bd is per-partition on pred^T so it
            # folds into the copy as an Identity bias — this also kills the
            # pathologically slow K=1 bd matmul (~630ns) from the chain.
            nc.scalar.activation(preds_sb[:, d * B:(d + 1) * B], pp[:],
                                 AF.Identity, bias=bd_sb[:])
            if d < n_out - 1:
                nc.scalar.activation(xd_sb[0:F_DIM, :], pp[:],
                                     AF.Identity, bias=bd_sb[:])
                return xd_sb
            return None

        def alloc_z():
            """Gate order f, i, g, o; g and o split into two 256-col chunks
            in separate PSUM tiles (full [128,512] banks, first 256 cols
            used — half-bank tiles would share banks and the bank-overlap
            tracker serializes reads against the bank-mate's writes)."""
            zf = zpool.tile([128, 512], F32, name="zf", tag="zf")
            zi = zpool.tile([128, 512], F32, name="zi", tag="zi")
            zg = [zspool.tile([128, 512], F32, name="zg%d" % s, tag="zg")[:, 0:256]
                  for s in (0, 1)]
            zo = [zspool.tile([128, 512], F32, name="zo%d" % s, tag="zo")[:, 0:256]
                  for s in (0, 1)]
            return (zf, zi, zg, zo)

        def z_layout(zs):
            """Block order g0, g1, f, i, o0, o1: the g tiles close FIRST in
            h@Wr so the long c-chain (tanh_g -> t1 -> c -> tanh_c) starts
            ~3.5us before the o tiles close; after o-close only sig_o and the
            h multiply remain before the transposes."""
            zf, zi, zg, zo = zs
            return ((2, zg[0], 0, 256), (2, zg[1], 256, 256),
                    (0, zf, 0, 512), (1, zi, 0, 512),
                    (3, zo[0], 0, 256), (3, zo[1], 256, 256))

        def emit_xwk(zs, x_sb, start, stop, blocks=None):
            """x @ Wk (+b); opens the PSUM groups when start=True (warmup)
            or closes them when stop=True (decode, where it comes last).
            `blocks` selects a subset of z_layout entries (warmup emits the
            o blocks separately: they wait on the previous step's sig_o
            PSUM-slot release, and fillers must cover that window)."""
            lay = z_layout(zs)
            if blocks is not None:
                lay = [lay[b] for b in blocks]
            for blk, z, lo, n in lay:
                for half in (0, 1):
                    o = (half * 4 + blk) * 512 + lo
                    nc.tensor.matmul(z[half * 64:(half + 1) * 64, :],
                                     x_sb[:], wk_sb[:, o:o + n],
                                     start=start, stop=stop)

        def emit_hwr(zs, xwk_first):
            """h @ Wr — lo/hi column-tile pairs emitted adjacently; k-chunks
            ordered by hT readiness. When xwk_first is False (decode), this
            opens the groups (except zf, opened by keep_warm) and leaves
            them open for the trailing x@Wk."""
            zf = zs[0]
            for blk, z, lo, n in z_layout(zs):
                for ki, k in enumerate((0, 4, 1, 5, 2, 6, 3, 7)):
                    stop = xwk_first and k == NK - 1
                    for half in (0, 1):
                        # keep_warm only opened zf's lo half (partitions 0:64)
                        start = ((not xwk_first) and ki == 0
                                 and not (z is zf and half == 0))
                        o = (k * 8 + half * 4 + blk) * 512 + lo
                        nc.tensor.matmul(
                            z[half * 64:(half + 1) * 64, :],
                            hT_sl(k), wr_sb[:, o:o + n],
                            start=start, stop=stop)

        def lstm_rest(zs, first):
            """Gate math, ordered for minimal o-close -> h latency.

            ScalarE FIFO order: sig_f, sig_i, tanh_g0/1 (fire as their z
            tiles close mid-h@Wr), then tanh_c0/1 (c is ready early: it only
            needs f,i,g), then sig_o0/1 LAST (o tiles close last). DVE FIFO:
            c-chain first, the two h multiplies last. Gates are bf16 so
            ScalarE runs at 2x accel and the h multiply hits the DVE 2x_1P
            mode -- the tail after the last o matmul is just sig_o (bf16)
            + h-mult instead of a 4-deep serialized f32 chain."""
            zf, zi, zg, zo = zs
            c_prev = state["c"]
            # tanh_g first on ScalarE: the g tiles close first in h@Wr, and
            # the c-chain hangs off tanh_g -- emitting sig_f/sig_i ahead of
            # them would FIFO-block tanh_g until the f/i tiles close.
            tanh_gs = []
            for s in (0, 1):
                tg = gpool.tile([128, 256], BF16, tag="tanh_g%d" % s,
                                name="tanh_g")
                nc.scalar.activation(tg[:], zg[s][:], AF.Tanh)
                tanh_gs.append(tg)
            sig_f = None
            if not first:
                sig_f = gpool.tile([128, 512], BF16, tag="sig_f", name="sig_f")
                for s in (0, 1):
                    nc.scalar.activation(sig_f[:, s * 256:(s + 1) * 256],
                                         zf[:, s * 256:(s + 1) * 256],
                                         AF.Sigmoid)
            # sig_i split in halves: t1_0 only needs the low half, so it can
            # start one ACT earlier (sig_i fires at i-close, the last of the
            # f/i/g tiles, and heads the critical c-chain).
            sig_i = gpool.tile([128, 512], BF16, tag="sig_i", name="sig_i")
            for s in (0, 1):
                nc.scalar.activation(sig_i[:, s * 256:(s + 1) * 256],
                                     zi[:, s * 256:(s + 1) * 256], AF.Sigmoid)
            # DVE c-chain (h multiplies are emitted after, so they don't
            # block the c ops in the DVE FIFO)
            c_new = []
            for s in (0, 1):
                sl = slice(s * 256, (s + 1) * 256)
                cs = cpool.tile([128, 256], BF16, tag="c%d" % s, name="c")
                if first:
                    nc.vector.tensor_tensor(cs[:], sig_i[:, sl], tanh_gs[s][:],
                                            Alu.mult)
                else:
                    t1 = gpool.tile([128, 256], BF16, tag="t1_%d" % s,
                                    name="t1")
                    nc.vector.tensor_tensor(t1[:], sig_i[:, sl], tanh_gs[s][:],
                                            Alu.mult)
                    nc.vector.tensor_tensor(cs[:], sig_f[:, sl], c_prev[s][:],
                                            Alu.mult)
                    nc.vector.tensor_tensor(cs[:], cs[:], t1[:], Alu.add)
                c_new.append(cs)
            # Tail, interleaved per half: tanh_c0, sig_o0 (h0's inputs) fire
            # before tanh_c1/sig_o1 on the Scalar FIFO, so h0 -- and with it
            # transpose pair 0 and the next step's first k-chunks -- is ready
            # two ACTs earlier. tanh_c before sig_o within each half because
            # c is ready well before the o tiles close.
            tanh_cs, sig_os = [], []
            for s in (0, 1):
                tc_s = gpool.tile([128, 256], BF16, tag="tanh_c%d" % s,
                                  name="tanh_c")
                nc.scalar.activation(tc_s[:], c_new[s][:], AF.Tanh)
                tanh_cs.append(tc_s)
                so = gpool.tile([128, 256], BF16, tag="sig_o%d" % s,
                                name="sig_o")
                nc.scalar.activation(so[:], zo[s][:], AF.Sigmoid)
                sig_os.append(so)
            h_new = []
            for s in (0, 1):
                hs = hpool.tile([128, 256], BF16, tag="h%d" % s, name="h")
                nc.vector.tensor_tensor(hs[:], sig_os[s][:], tanh_cs[s][:],
                                        Alu.mult)
                h_new.append(hs)
            state["h0"], state["h1"], state["c"] = h_new[0], h_new[1], c_new

        # ---- warmup ----
        for t in range(n_warm):
            x_sb = xpool.tile([F_DIM + 1, B], BF16)
            nc.sync.dma_start(x_sb[:], xt_ext[t])
            zs = alloc_z()
            if t == 0:
                emit_xwk(zs, x_sb, start=True, stop=True)
            else:
                # No fillers: the x@Wk pairs of this step plus the previous
                # step's transposes are the PE work that covers the tail of
                # the previous step's gate chain.
                emit_xwk(zs, x_sb, start=True, stop=False, blocks=(0, 1, 2, 3))
                emit_xwk(zs, x_sb, start=True, stop=False, blocks=(4, 5))
                state["hTa"] = hTpool.tile([128, 4 * B], BF16, name="hTa", tag="hTa")
                state["hTb"] = hTpool.tile([128, 4 * B], BF16, name="hTb", tag="hTb")
                transposes([0, 1])
                transposes([2, 3])
                emit_hwr(zs, xwk_first=True)
            lstm_rest(zs, first=(t == 0))

        # ---- decode: h@Wr first, x@Wk last, so the pred -> x_dec chain
        # hides under the recurrent matmuls ----
        for d in range(n_dec):
            zs = alloc_z()
            keep_warm(zs, 3, start=True)
            state["hTa"] = hTpool.tile([128, 4 * B], BF16, name="hTa", tag="hTa")
            state["hTb"] = hTpool.tile([128, 4 * B], BF16, name="hTb", tag="hTb")
            transposes([0, 1])
            transposes([2, 3])
            keep_warm(zs, 2)
            xd = pred_block(d)
            emit_hwr(zs, xwk_first=False)
            emit_xwk(zs, xd, start=False, stop=True)
            lstm_rest(zs, False)
        state["hTa"] = hTpool.tile([128, 4 * B], BF16, name="hTa", tag="hTa")
        state["hTb"] = hTpool.tile([128, 4 * B], BF16, name="hTb", tag="hTb")
        transposes([0, 1])
        transposes([2, 3])
        pred_block(n_out - 1)

        nc.sync.dma_start(out_ext[:], preds_sb[:])

    nc.finalize()
    _NC_CACHE[key] = nc
    return nc


def _prep_core_inputs(inputs, Wk, Wr, b, Wd, bd, n_warm, n_out):
    """Host-side reshaping/sharding. Returns list of 8 input dicts."""
    bf = lambda a: np.ascontiguousarray(a).astype(ml_dtypes.bfloat16)
    perm = np.array([g * UNITS + hh * 512 + k
                     for hh in (0, 1) for g in GATES for k in range(512)])
    Wk_aug = np.concatenate([Wk, b[None, :]], 0)[:, perm]        # [65, 4096]
    Wr_p = Wr[:, perm]                                           # [1024, 4096]
    wr_dev = bf(np.stack([Wr_p[k * 128:(k + 1) * 128] for k in range(NK)],
                         1).reshape(128, -1))
    wk_dev = bf(Wk_aug)
    wd_dev = bf(np.stack([Wd[k * 128:(k + 1) * 128] for k in range(NK)],
                         1).reshape(128, -1))
    bd_dev = np.ascontiguousarray(bd[:, None]).astype(np.float32)

    in_maps = []
    for c in range(N_CORES):
        xs = inputs[c * B:(c + 1) * B, :n_warm]                  # [64, T, F]
        xt = xs.transpose(1, 2, 0)                               # [T, F, 64]
        xt_aug = np.concatenate(
            [xt, np.ones((n_warm, 1, B), np.float32)], 1)        # [T, 65, 64]
        in_maps.append({
            "xt": bf(xt_aug), "wr": wr_dev, "wk": wk_dev,
            "wd": wd_dev, "bd": bd_dev,
        })
    return in_maps


def kernel(inputs, Wk, Wr, b, Wd, bd, out_steps):
    inputs = np.asarray(inputs, np.float32)
    Wk = np.asarray(Wk, np.float32)
    Wr = np.asarray(Wr, np.float32)
    b = np.asarray(b, np.float32)
    Wd = np.asarray(Wd, np.float32)
    bd = np.asarray(bd, np.float32)
    n_out = int(out_steps)
    n_warm = inputs.shape[1]

    nc = _build(n_warm, n_out)
    in_maps = _prep_core_inputs(inputs, Wk, Wr, b, Wd, bd, n_warm, n_out)
    res = run_bass_kernel_spmd(nc, in_maps, core_ids=list(range(N_CORES)))

    out = np.empty((B_FULL, n_out, F_DIM), np.float32)
    for c in range(N_CORES):
        o = res.results[c]["out"].reshape(F_DIM, n_out, B)       # [F, t, b]
        out[c * B:(c + 1) * B] = o.transpose(2, 1, 0)
    return out



# revision 12
# speedup vs baseline: 1.1931x; 1.1931x over previous
# Trainium2 Bass kernel for nn_AutoRegressive (LSTM warmup + autoregressive decode).
#
# Problem: B=512, T=128, F=64, UNITS=1024, OUT_STEPS=32.
#   warmup: 128 sequential LSTM steps over inputs, keep final (h, c)
#   decode: pred = h @ Wd + bd, feed pred back as x for 31 more steps
#   output: [B, 32, F]
#
# Strategy: pure 8-way data parallelism on the batch axis (64 rows/core),
# weights replicated, zero cross-core communication. Per step the dominant
# matmul z = x @ Wk + h @ Wr is computed with h^T-stationary matmuls
# (lhsT = h^T[k-chunk] [128, 64]) streaming Wr columns. Because the local
# batch is 64 (< 128 array columns), two matmuls are column-tiled at
# tile_position (0,0)/(0,64) to process the lo/hi unit-halves of each gate
# concurrently (emitted adjacently so the PE overlaps them), keeping the
# 128x128 PE array fully utilized.
# All matmul operands are bf16 (PSUM accumulates f32); gates/state are f32.
# h -> h^T via 4 PE transposes per step; each transpose PAIR has its own
# PSUM bank and its own hT SBUF tile (hTa/hTb), and the next step's k-loop
# is ordered by chunk readiness so the pair-0 copy unblocks half of it
# while pair 1 is in flight. The g and o gates are split into 2x256-col
# PSUM tiles so the c/h gate chain pipelines with the matmul tail; zero-
# accumulate filler matmuls absorb the remaining PE wait windows (keeps
# the HAM clock gate at 8/8). Decode runs h@Wr first and x@Wk last so the
# pred -> x_dec chain hides under the matmuls; pred copies run on ScalarE
# with bd folded in as a per-partition Identity bias. Bias b is folded
# into an augmented ones-row of x / extra row of Wk on the host.
# Measured: 1.811 ms exec on hardware, rel err 3.4e-3 vs the reference.
import os
import sys

sys.path.insert(0, "/opt/trn_rl_repo")

import numpy as np
import ml_dtypes

import concourse.bass as bass
import concourse.mybir as mybir
import concourse.tile as tile
from concourse import bacc
from concourse.bass_utils import run_bass_kernel_spmd
from concourse.masks import make_identity
from contextlib import ExitStack

F32, BF16 = mybir.dt.float32, mybir.dt.bfloat16
AF = mybir.ActivationFunctionType
Alu = mybir.AluOpType

B_FULL, T_FULL, F_DIM, UNITS = 512, 128, 64, 1024
N_CORES = 8
B = B_FULL // N_CORES          # 64 local batch rows
NK = UNITS // 128              # 8 k-chunks of the recurrent contraction
GATES = [1, 0, 2, 3]           # processing order f,i,g,o (orig packing i,f,g,o)

_NC_CACHE = {}


def _build(n_warm: int, n_out: int):
    """Build the per-core Bass program. n_out = number of predictions (32)."""
    key = (n_warm, n_out)
    if key in _NC_CACHE:
        return _NC_CACHE[key]

    n_dec = n_out - 1  # LSTM steps in decode phase

    nc = bacc.Bacc("TRN2", target_bir_lowering=False, debug=False,
                   num_devices=N_CORES)
    xt_ext = nc.dram_tensor("xt", [n_warm, F_DIM + 1, B], BF16,
                            kind="ExternalInput")
    wr_ext = nc.dram_tensor("wr", [128, NK * 8 * 512], BF16,
                            kind="ExternalInput")
    wk_ext = nc.dram_tensor("wk", [F_DIM + 1, 8 * 512], BF16,
                            kind="ExternalInput")
    wd_ext = nc.dram_tensor("wd", [128, NK * F_DIM], BF16,
                            kind="ExternalInput")
    bd_ext = nc.dram_tensor("bd", [F_DIM, 1], F32, kind="ExternalInput")
    out_ext = nc.dram_tensor("out", [F_DIM, n_out * B], F32,
                             kind="ExternalOutput")

    with ExitStack() as ctx:
        tc = ctx.enter_context(tile.TileContext(nc))
        wpool = ctx.enter_context(tc.tile_pool(name="w", bufs=1))
        xpool = ctx.enter_context(tc.tile_pool(name="x", bufs=3))
        hTpool = ctx.enter_context(tc.tile_pool(name="hT", bufs=2))
        hpool = ctx.enter_context(tc.tile_pool(name="h", bufs=3))
        cpool = ctx.enter_context(tc.tile_pool(name="c", bufs=3))
        gpool = ctx.enter_context(tc.tile_pool(name="g", bufs=3))
        xdpool = ctx.enter_context(tc.tile_pool(name="xd", bufs=2))
        zpool = ctx.enter_context(tc.tile_pool(name="z", bufs=1, space="PSUM"))
        zspool = ctx.enter_context(tc.tile_pool(name="zs", bufs=2, space="PSUM"))
        tpool = ctx.enter_context(tc.tile_pool(name="tp", bufs=1, space="PSUM"))

        wr_sb = wpool.tile([128, NK * 8 * 512], BF16)
        nc.sync.dma_start(wr_sb[:], wr_ext[:])
        wk_sb = wpool.tile([F_DIM + 1, 8 * 512], BF16)
        nc.sync.dma_start(wk_sb[:], wk_ext[:])
        wd_sb = wpool.tile([128, NK * F_DIM], BF16)
        nc.sync.dma_start(wd_sb[:], wd_ext[:])
        bd_sb = wpool.tile([F_DIM, 1], F32)
        nc.sync.dma_start(bd_sb[:], bd_ext[:])
        identb = wpool.tile([128, 128], BF16)
        make_identity(nc, identb[:])
        ones_sb = wpool.tile([1, B], BF16)
        nc.vector.memset(ones_sb[:], 1.0)
        zeros_sb = wpool.tile([128, 512], BF16)
        nc.vector.memset(zeros_sb[:], 0.0)
        preds_sb = wpool.tile([F_DIM, n_out * B], F32)
        xd_sb = wpool.tile([F_DIM + 1, B], BF16)
        nc.vector.memset(xd_sb[F_DIM:F_DIM + 1, :], 1.0)

        state = {"h0": None, "h1": None, "c": None, "hT": None}
        # hT column layout: transpose of h[:, j*128:(j+1)*128] yields unit
        # chunks j (cols 0:64) and j+4 (cols 64:128); store them adjacently
        # so each transpose pair needs ONE contiguous DVE copy.
        HT_POS = {}
        for j in range(4):
            HT_POS[j] = 2 * j
            HT_POS[j + 4] = 2 * j + 1

        def hT_sl(k):
            p = HT_POS[k]
            t = state["hTa"] if p < 4 else state["hTb"]
            return t[:, (p % 4) * B:(p % 4 + 1) * B]

        def transposes(js):
            """h halves (bf16, batch-major split layout) -> hT chunks (bf16).

            transpose of h[:, j*128:(j+1)*128] yields unit chunks j and j+4
            side by side. Each pair gets its own PSUM bank and its own hT
            SBUF tile so the copy of pair 0 (chunks 0,4,1,5) unblocks the
            first half of the next step's k-loop while pair 1 is still in
            flight."""
            # Each transpose pair gets its OWN PSUM bank and its own hT
            # SBUF tile: the pair-0 copy (chunks 0,4,1,5) unblocks the first
            # half of the next step's k-loop while pair 1 and its copy are
            # still in flight — real work instead of filler in the
            # HAM-activity window.
            half = js[0] // 2
            tag = "tp0" if half == 0 else "tps"
            tps = tpool.tile([128, 1024], BF16, name=tag, tag=tag)[:, 0:256]
            hT = state["hTa"] if half == 0 else state["hTb"]
            for jj, j in enumerate(js):
                h_half = state["h0"] if j < 2 else state["h1"]
                nc.tensor.transpose(tps[:, jj * 128:(jj + 1) * 128],
                                    h_half[:, (j % 2) * 128:(j % 2 + 1) * 128],
                                    identb[:])
            nc.vector.tensor_copy(hT[:], tps[:])

        def keep_warm(zs, n, start=False):
            """Filler matmuls accumulating an all-zeros K=1 product into the
            live zf tile: numerically a no-op, but real PE activity (keeps
            the HAM clock gate at 8/8 across the per-step gate-chain tail)
            that writes a consumed tile (so DCE keeps it). With start=True
            the first one opens zf's group (decode, where x@Wk comes last)."""
            zf = zs[0]
            for i in range(n):
                nc.tensor.matmul(zf[0:64, :], wr_sb[:, 0:64], zeros_sb[:],
                                 start=(start and i == 0), stop=False)

        def pred_block(d):
            """pred_d^T = Wd^T @ h + bd from current hT; returns x_dec tile."""
            # share the pair-0 transpose bank: it is released right after
            # copy-a (early), so the pred matmuls start ~0.7us sooner than
            # waiting for copy-b's slot. Safe now that the pred copies run
            # on ScalarE (the old DVE-queue release stall is gone).
            pp = tpool.tile([F_DIM, 512], F32, name="pp", tag="tp0")[:, 0:B]
            for ki, k in enumerate((0, 4, 1, 5, 2, 6, 3, 7)):
                nc.tensor.matmul(pp[:], wd_sb[:, k * F_DIM:(k + 1) * F_DIM],
                                 hT_sl(k), start=(k == 0), stop=(ki == 7))
            # Copies on ScalarE (idle here, and off the DVE queue which is
            # busy with the hT copies); bd is per-partition on pred^T so it
            # folds into the copy as an Identity bias — this also kills the
            # pathologically slow K=1 bd matmul (~630ns) from the chain.
            nc.scalar.activation(preds_sb[:, d * B:(d + 1) * B], pp[:],
                                 AF.Identity, bias=bd_sb[:])
            if d < n_out - 1:
                nc.scalar.activation(xd_sb[0:F_DIM, :], pp[:],
                                     AF.Identity, bias=bd_sb[:])
                return xd_sb
            return None

        def alloc_z():
            """Gate order f, i, g, o; g and o split into two 256-col chunks
            in separate PSUM tiles (full [128,512] banks, first 256 cols
            used — half-bank tiles would share banks and the bank-overlap
            tracker serializes reads against the bank-mate's writes)."""
            zf = zpool.tile([128, 512], F32, name="zf", tag="zf")
            zi = zpool.tile([128, 512], F32, name="zi", tag="zi")
            zg = [zspool.tile([128, 512], F32, name="zg%d" % s, tag="zg")[:, 0:256]
                  for s in (0, 1)]
            zo = [zspool.tile([128, 512], F32, name="zo%d" % s, tag="zo")[:, 0:256]
                  for s in (0, 1)]
            return (zf, zi, zg, zo)

        def z_layout(zs):
            """Block order g0, g1, f, i, o0, o1: the g tiles close FIRST in
            h@Wr so the long c-chain (tanh_g -> t1 -> c -> tanh_c) starts
            ~3.5us before the o tiles close; after o-close only sig_o and the
            h multiply remain before the transposes."""
            zf, zi, zg, zo = zs
            return ((2, zg[0], 0, 256), (2, zg[1], 256, 256),
                    (0, zf, 0, 512), (1, zi, 0, 512),
                    (3, zo[0], 0, 256), (3, zo[1], 256, 256))

        def emit_xwk(zs, x_sb, start, stop, blocks=None):
            """x @ Wk (+b); opens the PSUM groups when start=True (warmup)
            or closes them when stop=True (decode, where it comes last).
            `blocks` selects a subset of z_layout entries (warmup emits the
            o blocks separately: they wait on the previous step's sig_o
            PSUM-slot release, and fillers must cover that window)."""
            lay = z_layout(zs)
            if blocks is not None:
                lay = [lay[b] for b in blocks]
            for blk, z, lo, n in lay:
                for half in (0, 1):
                    o = (half * 4 + blk) * 512 + lo
                    nc.tensor.matmul(z[half * 64:(half + 1) * 64, :],
                                     x_sb[:], wk_sb[:, o:o + n],
                                     start=start, stop=stop)

        def emit_hwr_xwk_interleaved(zs, x_sb):
            """Decode: h@Wr with each tile CLOSED by its x@Wk pair right
            after that tile's k-loop. Tiles then close in block order (g
            first) ~1us apart, so the gate chain pipelines during h@Wr
            exactly like warmup, instead of bunching after a trailing
            x@Wk. Groups are opened here (except zf's lo half, opened by
            keep_warm)."""
            zf = zs[0]
            for blk, z, lo, n in z_layout(zs):
                for ki, k in enumerate((0, 4, 1, 5, 2, 6, 3, 7)):
                    for half in (0, 1):
                        start = (ki == 0 and not (z is zf and half == 0))
                        o = (k * 8 + half * 4 + blk) * 512 + lo
                        nc.tensor.matmul(
                            z[half * 64:(half + 1) * 64, :],
                            hT_sl(k), wr_sb[:, o:o + n],
                            start=start, stop=False)
                for half in (0, 1):
                    o = (half * 4 + blk) * 512 + lo
                    nc.tensor.matmul(z[half * 64:(half + 1) * 64, :],
                                     x_sb[:], wk_sb[:, o:o + n],
                                     start=False, stop=True)

        def emit_hwr(zs, xwk_first):
            """h @ Wr — lo/hi column-tile pairs emitted adjacently; k-chunks
            ordered by hT readiness. When xwk_first is False (decode), this
            opens the groups (except zf, opened by keep_warm) and leaves
            them open for the trailing x@Wk."""
            zf = zs[0]
            for blk, z, lo, n in z_layout(zs):
                for ki, k in enumerate((0, 4, 1, 5, 2, 6, 3, 7)):
                    stop = xwk_first and k == NK - 1
                    for half in (0, 1):
                        # keep_warm only opened zf's lo half (partitions 0:64)
                        start = ((not xwk_first) and ki == 0
                                 and not (z is zf and half == 0))
                        o = (k * 8 + half * 4 + blk) * 512 + lo
                        nc.tensor.matmul(
                            z[half * 64:(half + 1) * 64, :],
                            hT_sl(k), wr_sb[:, o:o + n],
                            start=start, stop=stop)

        def lstm_rest(zs, first):
            """Gate math, ordered for minimal o-close -> h latency.

            ScalarE FIFO order: sig_f, sig_i, tanh_g0/1 (fire as their z
            tiles close mid-h@Wr), then tanh_c0/1 (c is ready early: it only
            needs f,i,g), then sig_o0/1 LAST (o tiles close last). DVE FIFO:
            c-chain first, the two h multiplies last. Gates are bf16 so
            ScalarE runs at 2x accel and the h multiply hits the DVE 2x_1P
            mode -- the tail after the last o matmul is just sig_o (bf16)
            + h-mult instead of a 4-deep serialized f32 chain."""
            zf, zi, zg, zo = zs
            c_prev = state["c"]
            # tanh_g first on ScalarE: the g tiles close first in h@Wr, and
            # the c-chain hangs off tanh_g -- emitting sig_f/sig_i ahead of
            # them would FIFO-block tanh_g until the f/i tiles close.
            tanh_gs = []
            for s in (0, 1):
                tg = gpool.tile([128, 256], BF16, tag="tanh_g%d" % s,
                                name="tanh_g")
                nc.scalar.activation(tg[:], zg[s][:], AF.Tanh)
                tanh_gs.append(tg)
            sig_f = None
            if not first:
                sig_f = gpool.tile([128, 512], BF16, tag="sig_f", name="sig_f")
                nc.scalar.activation(sig_f[:], zf[:], AF.Sigmoid)
            sig_i = gpool.tile([128, 512], BF16, tag="sig_i", name="sig_i")
            nc.scalar.activation(sig_i[:], zi[:], AF.Sigmoid)
            # DVE c-chain (h multiplies are emitted after, so they don't
            # block the c ops in the DVE FIFO)
            c_new = []
            for s in (0, 1):
                sl = slice(s * 256, (s + 1) * 256)
                cs = cpool.tile([128, 256], BF16, tag="c%d" % s, name="c")
                if first:
                    nc.vector.tensor_tensor(cs[:], sig_i[:, sl], tanh_gs[s][:],
                                            Alu.mult)
                else:
                    t1 = gpool.tile([128, 256], BF16, tag="t1_%d" % s,
                                    name="t1")
                    nc.vector.tensor_tensor(t1[:], sig_i[:, sl], tanh_gs[s][:],
                                            Alu.mult)
                    nc.vector.tensor_tensor(cs[:], sig_f[:, sl], c_prev[s][:],
                                            Alu.mult)
                    nc.vector.tensor_tensor(cs[:], cs[:], t1[:], Alu.add)
                c_new.append(cs)
            # Tail, interleaved per half: tanh_c0, sig_o0 (h0's inputs) fire
            # before tanh_c1/sig_o1 on the Scalar FIFO, so h0 -- and with it
            # transpose pair 0 and the next step's first k-chunks -- is ready
            # two ACTs earlier. tanh_c before sig_o within each half because
            # c is ready well before the o tiles close.
            tanh_cs, sig_os = [], []
            for s in (0, 1):
                tc_s = gpool.tile([128, 256], BF16, tag="tanh_c%d" % s,
                                  name="tanh_c")
                nc.scalar.activation(tc_s[:], c_new[s][:], AF.Tanh)
                tanh_cs.append(tc_s)
                so = gpool.tile([128, 256], BF16, tag="sig_o%d" % s,
                                name="sig_o")
                nc.scalar.activation(so[:], zo[s][:], AF.Sigmoid)
                sig_os.append(so)
            h_new = []
            for s in (0, 1):
                hs = hpool.tile([128, 256], BF16, tag="h%d" % s, name="h")
                nc.vector.tensor_tensor(hs[:], sig_os[s][:], tanh_cs[s][:],
                                        Alu.mult)
                h_new.append(hs)
            state["h0"], state["h1"], state["c"] = h_new[0], h_new[1], c_new

        # ---- warmup ----
        for t in range(n_warm):
            x_sb = xpool.tile([F_DIM + 1, B], BF16)
            nc.sync.dma_start(x_sb[:], xt_ext[t])
            zs = alloc_z()
            if t == 0:
                emit_xwk(zs, x_sb, start=True, stop=True)
            else:
                # No fillers: the x@Wk pairs of this step plus the previous
                # step's transposes are the PE work that covers the tail of
                # the previous step's gate chain.
                emit_xwk(zs, x_sb, start=True, stop=False, blocks=(0, 1, 2, 3))
                emit_xwk(zs, x_sb, start=True, stop=False, blocks=(4, 5))
                state["hTa"] = hTpool.tile([128, 4 * B], BF16, name="hTa", tag="hTa")
                state["hTb"] = hTpool.tile([128, 4 * B], BF16, name="hTb", tag="hTb")
                transposes([0, 1])
                transposes([2, 3])
                emit_hwr(zs, xwk_first=True)
            lstm_rest(zs, first=(t == 0))

        # ---- decode: h@Wr first, x@Wk last, so the pred -> x_dec chain
        # hides under the recurrent matmuls ----
        for d in range(n_dec):
            zs = alloc_z()
            keep_warm(zs, 3, start=True)
            state["hTa"] = hTpool.tile([128, 4 * B], BF16, name="hTa", tag="hTa")
            state["hTb"] = hTpool.tile([128, 4 * B], BF16, name="hTb", tag="hTb")
            transposes([0, 1])
            transposes([2, 3])
            keep_warm(zs, 2)
            xd = pred_block(d)
            emit_hwr(zs, xwk_first=False)
            emit_xwk(zs, xd, start=False, stop=True)
            lstm_rest(zs, False)
        state["hTa"] = hTpool.tile([128, 4 * B], BF16, name="hTa", tag="hTa")
        state["hTb"] = hTpool.tile([128, 4 * B], BF16, name="hTb", tag="hTb")
        transposes([0, 1])
        transposes([2, 3])
        pred_block(n_out - 1)

        nc.sync.dma_start(out_ext[:], preds_sb[:])

    nc.finalize()
    _NC_CACHE[key] = nc
    return nc


def _prep_core_inputs(inputs, Wk, Wr, b, Wd, bd, n_warm, n_out):
    """Host-side reshaping/sharding. Returns list of 8 input dicts."""
    bf = lambda a: np.ascontiguousarray(a).astype(ml_dtypes.bfloat16)
    perm = np.array([g * UNITS + hh * 512 + k
                     for hh in (0, 1) for g in GATES for k in range(512)])
    Wk_aug = np.concatenate([Wk, b[None, :]], 0)[:, perm]        # [65, 4096]
    Wr_p = Wr[:, perm]                                           # [1024, 4096]
    wr_dev = bf(np.stack([Wr_p[k * 128:(k + 1) * 128] for k in range(NK)],
                         1).reshape(128, -1))
    wk_dev = bf(Wk_aug)
    wd_dev = bf(np.stack([Wd[k * 128:(k + 1) * 128] for k in range(NK)],
                         1).reshape(128, -1))
    bd_dev = np.ascontiguousarray(bd[:, None]).astype(np.float32)

    in_maps = []
    for c in range(N_CORES):
        xs = inputs[c * B:(c + 1) * B, :n_warm]                  # [64, T, F]
        xt = xs.transpose(1, 2, 0)                               # [T, F, 64]
        xt_aug = np.concatenate(
            [xt, np.ones((n_warm, 1, B), np.float32)], 1)        # [T, 65, 64]
        in_maps.append({
            "xt": bf(xt_aug), "wr": wr_dev, "wk": wk_dev,
            "wd": wd_dev, "bd": bd_dev,
        })
    return in_maps


def kernel(inputs, Wk, Wr, b, Wd, bd, out_steps):
    inputs = np.asarray(inputs, np.float32)
    Wk = np.asarray(Wk, np.float32)
    Wr = np.asarray(Wr, np.float32)
    b = np.asarray(b, np.float32)
    Wd = np.asarray(Wd, np.float32)
    bd = np.asarray(bd, np.float32)
    n_out = int(out_steps)
    n_warm = inputs.shape[1]

    nc = _build(n_warm, n_out)
    in_maps = _prep_core_inputs(inputs, Wk, Wr, b, Wd, bd, n_warm, n_out)
    res = run_bass_kernel_spmd(nc, in_maps, core_ids=list(range(N_CORES)))

    out = np.empty((B_FULL, n_out, F_DIM), np.float32)
    for c in range(N_CORES):
        o = res.results[c]["out"].reshape(F_DIM, n_out, B)       # [F, t, b]
        out[c * B:(c + 1) * B] = o.transpose(2, 1, 0)
    return out



# revision 13
# speedup vs baseline: 1.2297x; 1.0307x over previous
# Trainium2 Bass kernel for nn_AutoRegressive (LSTM warmup + autoregressive decode).
#
# Problem: B=512, T=128, F=64, UNITS=1024, OUT_STEPS=32.
#   warmup: 128 sequential LSTM steps over inputs, keep final (h, c)
#   decode: pred = h @ Wd + bd, feed pred back as x for 31 more steps
#   output: [B, 32, F]
#
# Strategy: pure 8-way data parallelism on the batch axis (64 rows/core),
# weights replicated, zero cross-core communication. Per step the dominant
# matmul z = x @ Wk + h @ Wr is computed with h^T-stationary matmuls
# (lhsT = h^T[k-chunk] [128, 64]) streaming Wr columns. Because the local
# batch is 64 (< 128 array columns), two matmuls are column-tiled at
# tile_position (0,0)/(0,64) to process the lo/hi unit-halves of each gate
# concurrently (emitted adjacently so the PE overlaps them), keeping the
# 128x128 PE array fully utilized.
# All matmul operands are bf16 (PSUM accumulates f32); gates/state are f32.
# h -> h^T via 4 PE transposes per step; each transpose PAIR has its own
# PSUM bank and its own hT SBUF tile (hTa/hTb), and the next step's k-loop
# is ordered by chunk readiness so the pair-0 copy unblocks half of it
# while pair 1 is in flight. The g and o gates are split into 2x256-col
# PSUM tiles so the c/h gate chain pipelines with the matmul tail; zero-
# accumulate filler matmuls absorb the remaining PE wait windows (keeps
# the HAM clock gate at 8/8). Decode runs h@Wr first and x@Wk last so the
# pred -> x_dec chain hides under the matmuls; pred copies run on ScalarE
# with bd folded in as a per-partition Identity bias. Bias b is folded
# into an augmented ones-row of x / extra row of Wk on the host.
# Measured: 1.811 ms exec on hardware, rel err 3.4e-3 vs the reference.
import os
import sys

sys.path.insert(0, "/opt/trn_rl_repo")

import numpy as np
import ml_dtypes

import concourse.bass as bass
import concourse.mybir as mybir
import concourse.tile as tile
from concourse import bacc
from concourse.bass_utils import run_bass_kernel_spmd
from concourse.masks import make_identity
from contextlib import ExitStack

F32, BF16 = mybir.dt.float32, mybir.dt.bfloat16
AF = mybir.ActivationFunctionType
Alu = mybir.AluOpType

B_FULL, T_FULL, F_DIM, UNITS = 512, 128, 64, 1024
N_CORES = 8
B = B_FULL // N_CORES          # 64 local batch rows
NK = UNITS // 128              # 8 k-chunks of the recurrent contraction
GATES = [1, 0, 2, 3]           # processing order f,i,g,o (orig packing i,f,g,o)

_NC_CACHE = {}


def _build(n_warm: int, n_out: int):
    """Build the per-core Bass program. n_out = number of predictions (32)."""
    key = (n_warm, n_out)
    if key in _NC_CACHE:
        return _NC_CACHE[key]

    n_dec = n_out - 1  # LSTM steps in decode phase

    nc = bacc.Bacc("TRN2", target_bir_lowering=False, debug=False,
                   num_devices=N_CORES)
    xt_ext = nc.dram_tensor("xt", [n_warm, F_DIM + 1, B], BF16,
                            kind="ExternalInput")
    wr_ext = nc.dram_tensor("wr", [128, NK * 8 * 512], BF16,
                            kind="ExternalInput")
    wk_ext = nc.dram_tensor("wk", [F_DIM + 1, 8 * 512], BF16,
                            kind="ExternalInput")
    wd_ext = nc.dram_tensor("wd", [128, NK * F_DIM], BF16,
                            kind="ExternalInput")
    bd_ext = nc.dram_tensor("bd", [F_DIM, 1], F32, kind="ExternalInput")
    out_ext = nc.dram_tensor("out", [F_DIM, n_out * B], F32,
                             kind="ExternalOutput")

    with ExitStack() as ctx:
        tc = ctx.enter_context(tile.TileContext(nc))
        wpool = ctx.enter_context(tc.tile_pool(name="w", bufs=1))
        xpool = ctx.enter_context(tc.tile_pool(name="x", bufs=3))
        hTpool = ctx.enter_context(tc.tile_pool(name="hT", bufs=2))
        hpool = ctx.enter_context(tc.tile_pool(name="h", bufs=3))
        cpool = ctx.enter_context(tc.tile_pool(name="c", bufs=3))
        gpool = ctx.enter_context(tc.tile_pool(name="g", bufs=3))
        xdpool = ctx.enter_context(tc.tile_pool(name="xd", bufs=2))
        zpool = ctx.enter_context(tc.tile_pool(name="z", bufs=1, space="PSUM"))
        zspool = ctx.enter_context(tc.tile_pool(name="zs", bufs=2, space="PSUM"))
        tpool = ctx.enter_context(tc.tile_pool(name="tp", bufs=1, space="PSUM"))

        wr_sb = wpool.tile([128, NK * 8 * 512], BF16)
        nc.sync.dma_start(wr_sb[:], wr_ext[:])
        wk_sb = wpool.tile([F_DIM + 1, 8 * 512], BF16)
        nc.sync.dma_start(wk_sb[:], wk_ext[:])
        wd_sb = wpool.tile([128, NK * F_DIM], BF16)
        nc.sync.dma_start(wd_sb[:], wd_ext[:])
        bd_sb = wpool.tile([F_DIM, 1], F32)
        nc.sync.dma_start(bd_sb[:], bd_ext[:])
        identb = wpool.tile([128, 128], BF16)
        make_identity(nc, identb[:])
        ones_sb = wpool.tile([1, B], BF16)
        nc.vector.memset(ones_sb[:], 1.0)
        zeros_sb = wpool.tile([128, 512], BF16)
        nc.vector.memset(zeros_sb[:], 0.0)
        preds_sb = wpool.tile([F_DIM, n_out * B], F32)
        xd_sb = wpool.tile([F_DIM + 1, B], BF16)
        nc.vector.memset(xd_sb[F_DIM:F_DIM + 1, :], 1.0)

        state = {"h0": None, "h1": None, "c": None, "hT": None}
        # hT column layout: transpose of h[:, j*128:(j+1)*128] yields unit
        # chunks j (cols 0:64) and j+4 (cols 64:128); store them adjacently
        # so each transpose pair needs ONE contiguous DVE copy.
        HT_POS = {}
        for j in range(4):
            HT_POS[j] = 2 * j
            HT_POS[j + 4] = 2 * j + 1

        def hT_sl(k):
            p = HT_POS[k]
            t = state["hTa"] if p < 4 else state["hTb"]
            return t[:, (p % 4) * B:(p % 4 + 1) * B]

        def transposes(js):
            """h halves (bf16, batch-major split layout) -> hT chunks (bf16).

            transpose of h[:, j*128:(j+1)*128] yields unit chunks j and j+4
            side by side. Each pair gets its own PSUM bank and its own hT
            SBUF tile so the copy of pair 0 (chunks 0,4,1,5) unblocks the
            first half of the next step's k-loop while pair 1 is still in
            flight."""
            # Each transpose pair gets its OWN PSUM bank and its own hT
            # SBUF tile: the pair-0 copy (chunks 0,4,1,5) unblocks the first
            # half of the next step's k-loop while pair 1 and its copy are
            # still in flight — real work instead of filler in the
            # HAM-activity window.
            half = js[0] // 2
            tag = "tp0" if half == 0 else "tps"
            tps = tpool.tile([128, 1024], BF16, name=tag, tag=tag)[:, 0:256]
            hT = state["hTa"] if half == 0 else state["hTb"]
            for jj, j in enumerate(js):
                h_half = state["h0"] if j < 2 else state["h1"]
                nc.tensor.transpose(tps[:, jj * 128:(jj + 1) * 128],
                                    h_half[:, (j % 2) * 128:(j % 2 + 1) * 128],
                                    identb[:])
            nc.vector.tensor_copy(hT[:], tps[:])

        def keep_warm(zs, n, start=False):
            """Filler matmuls accumulating an all-zeros K=1 product into the
            live zf tile: numerically a no-op, but real PE activity (keeps
            the HAM clock gate at 8/8 across the per-step gate-chain tail)
            that writes a consumed tile (so DCE keeps it). With start=True
            the first one opens zf's group (decode, where x@Wk comes last)."""
            zf = zs[0]
            for i in range(n):
                nc.tensor.matmul(zf[0:64, :], wr_sb[:, 0:64], zeros_sb[:],
                                 start=(start and i == 0), stop=False)

        def pred_block(d):
            """pred_d^T = Wd^T @ h + bd from current hT; returns x_dec tile."""
            # share the pair-0 transpose bank: it is released right after
            # copy-a (early), so the pred matmuls start ~0.7us sooner than
            # waiting for copy-b's slot. Safe now that the pred copies run
            # on ScalarE (the old DVE-queue release stall is gone).
            pp = tpool.tile([F_DIM, 512], F32, name="pp", tag="tp0")[:, 0:B]
            for ki, k in enumerate((0, 4, 1, 5, 2, 6, 3, 7)):
                nc.tensor.matmul(pp[:], wd_sb[:, k * F_DIM:(k + 1) * F_DIM],
                                 hT_sl(k), start=(k == 0), stop=(ki == 7))
            # Copies on ScalarE (idle here, and off the DVE queue which is
            # busy with the hT copies); bd is per-partition on pred^T so it
            # folds into the copy as an Identity bias — this also kills the
            # pathologically slow K=1 bd matmul (~630ns) from the chain.
            nc.scalar.activation(preds_sb[:, d * B:(d + 1) * B], pp[:],
                                 AF.Identity, bias=bd_sb[:])
            if d < n_out - 1:
                nc.scalar.activation(xd_sb[0:F_DIM, :], pp[:],
                                     AF.Identity, bias=bd_sb[:])
                return xd_sb
            return None

        def alloc_z():
            """Gate order f, i, g, o; g and o split into two 256-col chunks
            in separate PSUM tiles (full [128,512] banks, first 256 cols
            used — half-bank tiles would share banks and the bank-overlap
            tracker serializes reads against the bank-mate's writes)."""
            zf = zpool.tile([128, 512], F32, name="zf", tag="zf")
            zi = zpool.tile([128, 512], F32, name="zi", tag="zi")
            zg = [zspool.tile([128, 512], F32, name="zg%d" % s, tag="zg")[:, 0:256]
                  for s in (0, 1)]
            zo = [zspool.tile([128, 512], F32, name="zo%d" % s, tag="zo")[:, 0:256]
                  for s in (0, 1)]
            return (zf, zi, zg, zo)

        def z_layout(zs):
            """Block order g0, g1, f, i, o0, o1: the g tiles close FIRST in
            h@Wr so the long c-chain (tanh_g -> t1 -> c -> tanh_c) starts
            ~3.5us before the o tiles close; after o-close only sig_o and the
            h multiply remain before the transposes."""
            zf, zi, zg, zo = zs
            return ((2, zg[0], 0, 256), (2, zg[1], 256, 256),
                    (0, zf, 0, 512), (1, zi, 0, 512),
                    (3, zo[0], 0, 256), (3, zo[1], 256, 256))

        def emit_xwk(zs, x_sb, start, stop, blocks=None):
            """x @ Wk (+b); opens the PSUM groups when start=True (warmup)
            or closes them when stop=True (decode, where it comes last).
            `blocks` selects a subset of z_layout entries (warmup emits the
            o blocks separately: they wait on the previous step's sig_o
            PSUM-slot release, and fillers must cover that window)."""
            lay = z_layout(zs)
            if blocks is not None:
                lay = [lay[b] for b in blocks]
            for blk, z, lo, n in lay:
                for half in (0, 1):
                    o = (half * 4 + blk) * 512 + lo
                    nc.tensor.matmul(z[half * 64:(half + 1) * 64, :],
                                     x_sb[:], wk_sb[:, o:o + n],
                                     start=start, stop=stop)

        def emit_hwr_xwk_interleaved(zs, x_sb):
            """Decode: h@Wr with each tile CLOSED by its x@Wk pair right
            after that tile's k-loop. Tiles then close in block order (g
            first) ~1us apart, so the gate chain pipelines during h@Wr
            exactly like warmup, instead of bunching after a trailing
            x@Wk. Groups are opened here (except zf's lo half, opened by
            keep_warm)."""
            zf = zs[0]
            for blk, z, lo, n in z_layout(zs):
                for ki, k in enumerate((0, 4, 1, 5, 2, 6, 3, 7)):
                    for half in (0, 1):
                        start = (ki == 0 and not (z is zf and half == 0))
                        o = (k * 8 + half * 4 + blk) * 512 + lo
                        nc.tensor.matmul(
                            z[half * 64:(half + 1) * 64, :],
                            hT_sl(k), wr_sb[:, o:o + n],
                            start=start, stop=False)
                for half in (0, 1):
                    o = (half * 4 + blk) * 512 + lo
                    nc.tensor.matmul(z[half * 64:(half + 1) * 64, :],
                                     x_sb[:], wk_sb[:, o:o + n],
                                     start=False, stop=True)

        def emit_hwr(zs, xwk_first):
            """h @ Wr — lo/hi column-tile pairs emitted adjacently; k-chunks
            ordered by hT readiness. When xwk_first is False (decode), this
            opens the groups (except zf, opened by keep_warm) and leaves
            them open for the trailing x@Wk."""
            zf = zs[0]
            for blk, z, lo, n in z_layout(zs):
                for ki, k in enumerate((0, 4, 1, 5, 2, 6, 3, 7)):
                    stop = xwk_first and k == NK - 1
                    for half in (0, 1):
                        # keep_warm only opened zf's lo half (partitions 0:64)
                        start = ((not xwk_first) and ki == 0
                                 and not (z is zf and half == 0))
                        o = (k * 8 + half * 4 + blk) * 512 + lo
                        nc.tensor.matmul(
                            z[half * 64:(half + 1) * 64, :],
                            hT_sl(k), wr_sb[:, o:o + n],
                            start=start, stop=stop)

        def lstm_rest(zs, first):
            """Gate math, ordered for minimal o-close -> h latency.

            ScalarE FIFO order: sig_f, sig_i, tanh_g0/1 (fire as their z
            tiles close mid-h@Wr), then tanh_c0/1 (c is ready early: it only
            needs f,i,g), then sig_o0/1 LAST (o tiles close last). DVE FIFO:
            c-chain first, the two h multiplies last. Gates are bf16 so
            ScalarE runs at 2x accel and the h multiply hits the DVE 2x_1P
            mode -- the tail after the last o matmul is just sig_o (bf16)
            + h-mult instead of a 4-deep serialized f32 chain."""
            zf, zi, zg, zo = zs
            c_prev = state["c"]
            # tanh_g first on ScalarE: the g tiles close first in h@Wr, and
            # the c-chain hangs off tanh_g -- emitting sig_f/sig_i ahead of
            # them would FIFO-block tanh_g until the f/i tiles close.
            tanh_gs = []
            for s in (0, 1):
                tg = gpool.tile([128, 256], BF16, tag="tanh_g%d" % s,
                                name="tanh_g")
                nc.scalar.activation(tg[:], zg[s][:], AF.Tanh)
                tanh_gs.append(tg)
            sig_f = None
            if not first:
                sig_f = gpool.tile([128, 512], BF16, tag="sig_f", name="sig_f")
                nc.scalar.activation(sig_f[:], zf[:], AF.Sigmoid)
            sig_i = gpool.tile([128, 512], BF16, tag="sig_i", name="sig_i")
            nc.scalar.activation(sig_i[:], zi[:], AF.Sigmoid)
            # DVE c-chain (h multiplies are emitted after, so they don't
            # block the c ops in the DVE FIFO)
            c_new = []
            for s in (0, 1):
                sl = slice(s * 256, (s + 1) * 256)
                cs = cpool.tile([128, 256], BF16, tag="c%d" % s, name="c")
                if first:
                    nc.vector.tensor_tensor(cs[:], sig_i[:, sl], tanh_gs[s][:],
                                            Alu.mult)
                else:
                    t1 = gpool.tile([128, 256], BF16, tag="t1_%d" % s,
                                    name="t1")
                    nc.vector.tensor_tensor(t1[:], sig_i[:, sl], tanh_gs[s][:],
                                            Alu.mult)
                    nc.vector.tensor_tensor(cs[:], sig_f[:, sl], c_prev[s][:],
                                            Alu.mult)
                    nc.vector.tensor_tensor(cs[:], cs[:], t1[:], Alu.add)
                c_new.append(cs)
            # Tail, interleaved per half: tanh_c0, sig_o0 (h0's inputs) fire
            # before tanh_c1/sig_o1 on the Scalar FIFO, so h0 -- and with it
            # transpose pair 0 and the next step's first k-chunks -- is ready
            # two ACTs earlier. tanh_c before sig_o within each half because
            # c is ready well before the o tiles close.
            tanh_cs, sig_os = [], []
            for s in (0, 1):
                tc_s = gpool.tile([128, 256], BF16, tag="tanh_c%d" % s,
                                  name="tanh_c")
                nc.scalar.activation(tc_s[:], c_new[s][:], AF.Tanh)
                tanh_cs.append(tc_s)
                so = gpool.tile([128, 256], BF16, tag="sig_o%d" % s,
                                name="sig_o")
                nc.scalar.activation(so[:], zo[s][:], AF.Sigmoid)
                sig_os.append(so)
            h_new = []
            for s in (0, 1):
                hs = hpool.tile([128, 256], BF16, tag="h%d" % s, name="h")
                nc.vector.tensor_tensor(hs[:], sig_os[s][:], tanh_cs[s][:],
                                        Alu.mult)
                h_new.append(hs)
            state["h0"], state["h1"], state["c"] = h_new[0], h_new[1], c_new

        # ---- warmup ----
        for t in range(n_warm):
            x_sb = xpool.tile([F_DIM + 1, B], BF16)
            nc.sync.dma_start(x_sb[:], xt_ext[t])
            zs = alloc_z()
            if t == 0:
                emit_xwk(zs, x_sb, start=True, stop=True)
            else:
                # No fillers: the x@Wk pairs of this step plus the previous
                # step's transposes are the PE work that covers the tail of
                # the previous step's gate chain.
                emit_xwk(zs, x_sb, start=True, stop=False, blocks=(0, 1, 2, 3))
                emit_xwk(zs, x_sb, start=True, stop=False, blocks=(4, 5))
                state["hTa"] = hTpool.tile([128, 4 * B], BF16, name="hTa", tag="hTa")
                state["hTb"] = hTpool.tile([128, 4 * B], BF16, name="hTb", tag="hTb")
                transposes([0, 1])
                transposes([2, 3])
                emit_hwr(zs, xwk_first=True)
            lstm_rest(zs, first=(t == 0))

        # ---- decode: h@Wr first, x@Wk last, so the pred -> x_dec chain
        # hides under the recurrent matmuls ----
        for d in range(n_dec):
            zs = alloc_z()
            keep_warm(zs, 3, start=True)
            state["hTa"] = hTpool.tile([128, 4 * B], BF16, name="hTa", tag="hTa")
            state["hTb"] = hTpool.tile([128, 4 * B], BF16, name="hTb", tag="hTb")
            transposes([0, 1])
            transposes([2, 3])
            keep_warm(zs, 2)
            xd = pred_block(d)
            emit_hwr_xwk_interleaved(zs, xd)
            lstm_rest(zs, False)
        state["hTa"] = hTpool.tile([128, 4 * B], BF16, name="hTa", tag="hTa")
        state["hTb"] = hTpool.tile([128, 4 * B], BF16, name="hTb", tag="hTb")
        transposes([0, 1])
        transposes([2, 3])
        pred_block(n_out - 1)

        nc.sync.dma_start(out_ext[:], preds_sb[:])

    nc.finalize()
    _NC_CACHE[key] = nc
    return nc


def _prep_core_inputs(inputs, Wk, Wr, b, Wd, bd, n_warm, n_out):
    """Host-side reshaping/sharding. Returns list of 8 input dicts."""
    bf = lambda a: np.ascontiguousarray(a).astype(ml_dtypes.bfloat16)
    perm = np.array([g * UNITS + hh * 512 + k
                     for hh in (0, 1) for g in GATES for k in range(512)])
    Wk_aug = np.concatenate([Wk, b[None, :]], 0)[:, perm]        # [65, 4096]
    Wr_p = Wr[:, perm]                                           # [1024, 4096]
    wr_dev = bf(np.stack([Wr_p[k * 128:(k + 1) * 128] for k in range(NK)],
                         1).reshape(128, -1))
    wk_dev = bf(Wk_aug)
    wd_dev = bf(np.stack([Wd[k * 128:(k + 1) * 128] for k in range(NK)],
                         1).reshape(128, -1))
    bd_dev = np.ascontiguousarray(bd[:, None]).astype(np.float32)

    in_maps = []
    for c in range(N_CORES):
        xs = inputs[c * B:(c + 1) * B, :n_warm]                  # [64, T, F]
        xt = xs.transpose(1, 2, 0)                               # [T, F, 64]
        xt_aug = np.concatenate(
            [xt, np.ones((n_warm, 1, B), np.float32)], 1)        # [T, 65, 64]
        in_maps.append({
            "xt": bf(xt_aug), "wr": wr_dev, "wk": wk_dev,
            "wd": wd_dev, "bd": bd_dev,
        })
    return in_maps


def kernel(inputs, Wk, Wr, b, Wd, bd, out_steps):
    inputs = np.asarray(inputs, np.float32)
    Wk = np.asarray(Wk, np.float32)
    Wr = np.asarray(Wr, np.float32)
    b = np.asarray(b, np.float32)
    Wd = np.asarray(Wd, np.float32)
    bd = np.asarray(bd, np.float32)
    n_out = int(out_steps)
    n_warm = inputs.shape[1]

    nc = _build(n_warm, n_out)
    in_maps = _prep_core_inputs(inputs, Wk, Wr, b, Wd, bd, n_warm, n_out)
    res = run_bass_kernel_spmd(nc, in_maps, core_ids=list(range(N_CORES)))

    out = np.empty((B_FULL, n_out, F_DIM), np.float32)
    for c in range(N_CORES):
        o = res.results[c]["out"].reshape(F_DIM, n_out, B)       # [F, t, b]
        out[c * B:(c + 1) * B] = o.transpose(2, 1, 0)
    return out



# revision 16
# speedup vs baseline: 1.2540x; 1.0197x over previous
# Trainium2 Bass kernel for nn_AutoRegressive (LSTM warmup + autoregressive decode).
#
# Problem: B=512, T=128, F=64, UNITS=1024, OUT_STEPS=32.
#   warmup: 128 sequential LSTM steps over inputs, keep final (h, c)
#   decode: pred = h @ Wd + bd, feed pred back as x for 31 more steps
#   output: [B, 32, F]
#
# Strategy: pure 8-way data parallelism on the batch axis (64 rows/core),
# weights replicated, zero cross-core communication. Per step the dominant
# matmul z = x @ Wk + h @ Wr is computed with h^T-stationary matmuls
# (lhsT = h^T[k-chunk] [128, 64]) streaming Wr columns. Because the local
# batch is 64 (< 128 array columns), two matmuls are column-tiled at
# tile_position (0,0)/(0,64) to process the lo/hi unit-halves of each gate
# concurrently (emitted adjacently so the PE overlaps them), keeping the
# 128x128 PE array fully utilized.
# All matmul operands are bf16 (PSUM accumulates f32); gates/state are f32.
# h -> h^T via 4 PE transposes per step; each transpose PAIR has its own
# PSUM bank and its own hT SBUF tile (hTa/hTb), and the next step's k-loop
# is ordered by chunk readiness so the pair-0 copy unblocks half of it
# while pair 1 is in flight. The g and o gates are split into 2x256-col
# PSUM tiles so the c/h gate chain pipelines with the matmul tail; zero-
# accumulate filler matmuls absorb the remaining PE wait windows (keeps
# the HAM clock gate at 8/8). Decode runs h@Wr first and x@Wk last so the
# pred -> x_dec chain hides under the matmuls; pred copies run on ScalarE
# with bd folded in as a per-partition Identity bias. Bias b is folded
# into an augmented ones-row of x / extra row of Wk on the host.
# Measured: 1.811 ms exec on hardware, rel err 3.4e-3 vs the reference.
import os
import sys

sys.path.insert(0, "/opt/trn_rl_repo")

import numpy as np
import ml_dtypes

import concourse.bass as bass
import concourse.mybir as mybir
import concourse.tile as tile
from concourse import bacc
from concourse.bass_utils import run_bass_kernel_spmd
from concourse.masks import make_identity
from contextlib import ExitStack

F32, BF16 = mybir.dt.float32, mybir.dt.bfloat16
AF = mybir.ActivationFunctionType
Alu = mybir.AluOpType

B_FULL, T_FULL, F_DIM, UNITS = 512, 128, 64, 1024
N_CORES = 8
B = B_FULL // N_CORES          # 64 local batch rows
NK = UNITS // 128              # 8 k-chunks of the recurrent contraction
GATES = [1, 0, 2, 3]           # processing order f,i,g,o (orig packing i,f,g,o)

_NC_CACHE = {}


def _build(n_warm: int, n_out: int):
    """Build the per-core Bass program. n_out = number of predictions (32)."""
    key = (n_warm, n_out)
    if key in _NC_CACHE:
        return _NC_CACHE[key]

    n_dec = n_out - 1  # LSTM steps in decode phase

    nc = bacc.Bacc("TRN2", target_bir_lowering=False, debug=False,
                   num_devices=N_CORES)
    xt_ext = nc.dram_tensor("xt", [n_warm, F_DIM + 1, B], BF16,
                            kind="ExternalInput")
    wr_ext = nc.dram_tensor("wr", [128, NK * 8 * 512], BF16,
                            kind="ExternalInput")
    wk_ext = nc.dram_tensor("wk", [F_DIM + 1, 8 * 512], BF16,
                            kind="ExternalInput")
    wd_ext = nc.dram_tensor("wd", [128, NK * F_DIM], BF16,
                            kind="ExternalInput")
    bd_ext = nc.dram_tensor("bd", [F_DIM, 1], F32, kind="ExternalInput")
    out_ext = nc.dram_tensor("out", [F_DIM, n_out * B], F32,
                             kind="ExternalOutput")

    with ExitStack() as ctx:
        tc = ctx.enter_context(tile.TileContext(nc))
        wpool = ctx.enter_context(tc.tile_pool(name="w", bufs=1))
        xpool = ctx.enter_context(tc.tile_pool(name="x", bufs=3))
        hTpool = ctx.enter_context(tc.tile_pool(name="hT", bufs=2))
        hpool = ctx.enter_context(tc.tile_pool(name="h", bufs=3))
        cpool = ctx.enter_context(tc.tile_pool(name="c", bufs=3))
        gpool = ctx.enter_context(tc.tile_pool(name="g", bufs=3))
        xdpool = ctx.enter_context(tc.tile_pool(name="xd", bufs=2))
        zpool = ctx.enter_context(tc.tile_pool(name="z", bufs=1, space="PSUM"))
        zspool = ctx.enter_context(tc.tile_pool(name="zs", bufs=2, space="PSUM"))
        tpool = ctx.enter_context(tc.tile_pool(name="tp", bufs=1, space="PSUM"))

        wr_sb = wpool.tile([128, NK * 8 * 512], BF16)
        nc.sync.dma_start(wr_sb[:], wr_ext[:])
        wk_sb = wpool.tile([F_DIM + 1, 8 * 512], BF16)
        nc.sync.dma_start(wk_sb[:], wk_ext[:])
        wd_sb = wpool.tile([128, NK * F_DIM], BF16)
        nc.sync.dma_start(wd_sb[:], wd_ext[:])
        bd_sb = wpool.tile([F_DIM, 1], F32)
        nc.sync.dma_start(bd_sb[:], bd_ext[:])
        identb = wpool.tile([128, 128], BF16)
        make_identity(nc, identb[:])
        ones_sb = wpool.tile([1, B], BF16)
        nc.vector.memset(ones_sb[:], 1.0)
        zeros_sb = wpool.tile([128, 512], BF16)
        nc.vector.memset(zeros_sb[:], 0.0)
        preds_sb = wpool.tile([F_DIM, n_out * B], F32)
        xd_sb = wpool.tile([F_DIM + 1, B], BF16)
        nc.vector.memset(xd_sb[F_DIM:F_DIM + 1, :], 1.0)

        state = {"h0": None, "h1": None, "c": None, "hT": None}
        # hT column layout: transpose of h[:, j*128:(j+1)*128] yields unit
        # chunks j (cols 0:64) and j+4 (cols 64:128); store them adjacently
        # so each transpose pair needs ONE contiguous DVE copy.
        HT_POS = {}
        for j in range(4):
            HT_POS[j] = 2 * j
            HT_POS[j + 4] = 2 * j + 1

        def hT_sl(k):
            p = HT_POS[k]
            t = state["hTa"] if p < 4 else state["hTb"]
            return t[:, (p % 4) * B:(p % 4 + 1) * B]

        def transposes(js):
            """h halves (bf16, batch-major split layout) -> hT chunks (bf16).

            transpose of h[:, j*128:(j+1)*128] yields unit chunks j and j+4
            side by side. Each pair gets its own PSUM bank and its own hT
            SBUF tile so the copy of pair 0 (chunks 0,4,1,5) unblocks the
            first half of the next step's k-loop while pair 1 is still in
            flight."""
            # Each transpose pair gets its OWN PSUM bank and its own hT
            # SBUF tile: the pair-0 copy (chunks 0,4,1,5) unblocks the first
            # half of the next step's k-loop while pair 1 and its copy are
            # still in flight — real work instead of filler in the
            # HAM-activity window.
            # Both pairs share ONE PSUM bank (tag tp0): pair 1 waits for the
            # pair-0 copy, which has slack; the freed bank buys zo a third
            # buffer so next-step x@Wk-o pairs never wait the sig_o handoff.
            half = js[0] // 2
            name = "tp0" if half == 0 else "tps"
            tps = tpool.tile([128, 1024], BF16, name=name, tag="tp0")[:, 0:256]
            hT = state["hTa"] if half == 0 else state["hTb"]
            for jj, j in enumerate(js):
                h_half = state["h0"] if j < 2 else state["h1"]
                nc.tensor.transpose(tps[:, jj * 128:(jj + 1) * 128],
                                    h_half[:, (j % 2) * 128:(j % 2 + 1) * 128],
                                    identb[:])
            nc.vector.tensor_copy(hT[:], tps[:])

        def keep_warm(zs, n, start=False):
            """Filler matmuls accumulating an all-zeros K=1 product into the
            live zf tile: numerically a no-op, but real PE activity (keeps
            the HAM clock gate at 8/8 across the per-step gate-chain tail)
            that writes a consumed tile (so DCE keeps it). With start=True
            the first one opens zf's group (decode, where x@Wk comes last)."""
            zf = zs[0]
            for i in range(n):
                nc.tensor.matmul(zf[0:64, :], wr_sb[:, 0:64], zeros_sb[:],
                                 start=(start and i == 0), stop=False)

        def pred_block(d):
            """pred_d^T = Wd^T @ h + bd from current hT; returns x_dec tile."""
            # share the pair-0 transpose bank: it is released right after
            # copy-a (early), so the pred matmuls start ~0.7us sooner than
            # waiting for copy-b's slot. Safe now that the pred copies run
            # on ScalarE (the old DVE-queue release stall is gone).
            pp = tpool.tile([F_DIM, 512], F32, name="pp", tag="tp0")[:, 0:B]
            for ki, k in enumerate((0, 4, 1, 5, 2, 6, 3, 7)):
                nc.tensor.matmul(pp[:], wd_sb[:, k * F_DIM:(k + 1) * F_DIM],
                                 hT_sl(k), start=(k == 0), stop=(ki == 7))
            # Copies on ScalarE (idle here, and off the DVE queue which is
            # busy with the hT copies); bd is per-partition on pred^T so it
            # folds into the copy as an Identity bias — this also kills the
            # pathologically slow K=1 bd matmul (~630ns) from the chain.
            nc.scalar.activation(preds_sb[:, d * B:(d + 1) * B], pp[:],
                                 AF.Identity, bias=bd_sb[:])
            if d < n_out - 1:
                nc.scalar.activation(xd_sb[0:F_DIM, :], pp[:],
                                     AF.Identity, bias=bd_sb[:])
                return xd_sb
            return None

        def alloc_z():
            """Gate order f, i, g, o; g and o split into two 256-col chunks
            in separate PSUM tiles (full [128,512] banks, first 256 cols
            used — half-bank tiles would share banks and the bank-overlap
            tracker serializes reads against the bank-mate's writes)."""
            zf = zpool.tile([128, 512], F32, name="zf", tag="zf")
            zi = zpool.tile([128, 512], F32, name="zi", tag="zi")
            zg = [zspool.tile([128, 512], F32, name="zg%d" % s, tag="zg")[:, 0:256]
                  for s in (0, 1)]
            zo = [zspool.tile([128, 512], F32, name="zo%d" % s, tag="zo",
                              bufs=3)[:, 0:256]
                  for s in (0, 1)]
            return (zf, zi, zg, zo)

        def z_layout(zs):
            """Block order g0, g1, f, i, o0, o1: the g tiles close FIRST in
            h@Wr so the long c-chain (tanh_g -> t1 -> c -> tanh_c) starts
            ~3.5us before the o tiles close; after o-close only sig_o and the
            h multiply remain before the transposes."""
            zf, zi, zg, zo = zs
            return ((2, zg[0], 0, 256), (2, zg[1], 256, 256),
                    (0, zf, 0, 512), (1, zi, 0, 512),
                    (3, zo[0], 0, 256), (3, zo[1], 256, 256))

        def emit_xwk(zs, x_sb, start, stop, blocks=None):
            """x @ Wk (+b); opens the PSUM groups when start=True (warmup)
            or closes them when stop=True (decode, where it comes last).
            `blocks` selects a subset of z_layout entries (warmup emits the
            o blocks separately: they wait on the previous step's sig_o
            PSUM-slot release, and fillers must cover that window)."""
            lay = z_layout(zs)
            if blocks is not None:
                lay = [lay[b] for b in blocks]
            for blk, z, lo, n in lay:
                for half in (0, 1):
                    o = (half * 4 + blk) * 512 + lo
                    nc.tensor.matmul(z[half * 64:(half + 1) * 64, :],
                                     x_sb[:], wk_sb[:, o:o + n],
                                     start=start, stop=stop)

        def emit_hwr_xwk_interleaved(zs, x_sb):
            """Decode: h@Wr with each tile CLOSED by its x@Wk pair right
            after that tile's k-loop. Tiles then close in block order (g
            first) ~1us apart, so the gate chain pipelines during h@Wr
            exactly like warmup, instead of bunching after a trailing
            x@Wk. Groups are opened here (except zf's lo half, opened by
            keep_warm)."""
            zf = zs[0]
            for blk, z, lo, n in z_layout(zs):
                for ki, k in enumerate((0, 4, 1, 5, 2, 6, 3, 7)):
                    for half in (0, 1):
                        start = (ki == 0 and not (z is zf and half == 0))
                        o = (k * 8 + half * 4 + blk) * 512 + lo
                        nc.tensor.matmul(
                            z[half * 64:(half + 1) * 64, :],
                            hT_sl(k), wr_sb[:, o:o + n],
                            start=start, stop=False)
                for half in (0, 1):
                    o = (half * 4 + blk) * 512 + lo
                    nc.tensor.matmul(z[half * 64:(half + 1) * 64, :],
                                     x_sb[:], wk_sb[:, o:o + n],
                                     start=False, stop=True)

        def emit_hwr(zs, xwk_first):
            """h @ Wr — lo/hi column-tile pairs emitted adjacently; k-chunks
            ordered by hT readiness. When xwk_first is False (decode), this
            opens the groups (except zf, opened by keep_warm) and leaves
            them open for the trailing x@Wk."""
            zf = zs[0]
            for blk, z, lo, n in z_layout(zs):
                for ki, k in enumerate((0, 4, 1, 5, 2, 6, 3, 7)):
                    stop = xwk_first and k == NK - 1
                    for half in (0, 1):
                        # keep_warm only opened zf's lo half (partitions 0:64)
                        start = ((not xwk_first) and ki == 0
                                 and not (z is zf and half == 0))
                        o = (k * 8 + half * 4 + blk) * 512 + lo
                        nc.tensor.matmul(
                            z[half * 64:(half + 1) * 64, :],
                            hT_sl(k), wr_sb[:, o:o + n],
                            start=start, stop=stop)

        def lstm_rest(zs, first):
            """Gate math, ordered for minimal o-close -> h latency.

            ScalarE FIFO order: sig_f, sig_i, tanh_g0/1 (fire as their z
            tiles close mid-h@Wr), then tanh_c0/1 (c is ready early: it only
            needs f,i,g), then sig_o0/1 LAST (o tiles close last). DVE FIFO:
            c-chain first, the two h multiplies last. Gates are bf16 so
            ScalarE runs at 2x accel and the h multiply hits the DVE 2x_1P
            mode -- the tail after the last o matmul is just sig_o (bf16)
            + h-mult instead of a 4-deep serialized f32 chain."""
            zf, zi, zg, zo = zs
            c_prev = state["c"]
            # tanh_g first on ScalarE: the g tiles close first in h@Wr, and
            # the c-chain hangs off tanh_g -- emitting sig_f/sig_i ahead of
            # them would FIFO-block tanh_g until the f/i tiles close.
            tanh_gs = []
            for s in (0, 1):
                tg = gpool.tile([128, 256], BF16, tag="tanh_g%d" % s,
                                name="tanh_g")
                nc.scalar.activation(tg[:], zg[s][:], AF.Tanh)
                tanh_gs.append(tg)
            sig_f = None
            if not first:
                sig_f = gpool.tile([128, 512], BF16, tag="sig_f", name="sig_f")
                nc.scalar.activation(sig_f[:], zf[:], AF.Sigmoid)
            # sig_i heads the critical c-chain (zi closes last of f/i/g);
            # halving it lets t1_0 start one 256-wide ACT earlier.
            sig_i = gpool.tile([128, 512], BF16, tag="sig_i", name="sig_i")
            for s in (0, 1):
                nc.scalar.activation(sig_i[:, s * 256:(s + 1) * 256],
                                     zi[:, s * 256:(s + 1) * 256], AF.Sigmoid)
            # DVE c-chain (h multiplies are emitted after, so they don't
            # block the c ops in the DVE FIFO)
            c_new = []
            for s in (0, 1):
                sl = slice(s * 256, (s + 1) * 256)
                cs = cpool.tile([128, 256], BF16, tag="c%d" % s, name="c")
                if first:
                    nc.vector.tensor_tensor(cs[:], sig_i[:, sl], tanh_gs[s][:],
                                            Alu.mult)
                else:
                    t1 = gpool.tile([128, 256], BF16, tag="t1_%d" % s,
                                    name="t1")
                    nc.vector.tensor_tensor(t1[:], sig_i[:, sl], tanh_gs[s][:],
                                            Alu.mult)
                    nc.vector.tensor_tensor(cs[:], sig_f[:, sl], c_prev[s][:],
                                            Alu.mult)
                    nc.vector.tensor_tensor(cs[:], cs[:], t1[:], Alu.add)
                c_new.append(cs)
            # Tail, interleaved per half: tanh_c0, sig_o0 (h0's inputs) fire
            # before tanh_c1/sig_o1 on the Scalar FIFO, so h0 -- and with it
            # transpose pair 0 and the next step's first k-chunks -- is ready
            # two ACTs earlier. tanh_c before sig_o within each half because
            # c is ready well before the o tiles close.
            tanh_cs, sig_os = [], []
            for s in (0, 1):
                tc_s = gpool.tile([128, 256], BF16, tag="tanh_c%d" % s,
                                  name="tanh_c")
                nc.scalar.activation(tc_s[:], c_new[s][:], AF.Tanh)
                tanh_cs.append(tc_s)
                so = gpool.tile([128, 256], BF16, tag="sig_o%d" % s,
                                name="sig_o")
                nc.scalar.activation(so[:], zo[s][:], AF.Sigmoid)
                sig_os.append(so)
            h_new = []
            for s in (0, 1):
                hs = hpool.tile([128, 256], BF16, tag="h%d" % s, name="h")
                nc.vector.tensor_tensor(hs[:], sig_os[s][:], tanh_cs[s][:],
                                        Alu.mult)
                h_new.append(hs)
            state["h0"], state["h1"], state["c"] = h_new[0], h_new[1], c_new

        # ---- warmup ----
        for t in range(n_warm):
            x_sb = xpool.tile([F_DIM + 1, B], BF16)
            nc.sync.dma_start(x_sb[:], xt_ext[t])
            zs = alloc_z()
            if t == 0:
                emit_xwk(zs, x_sb, start=True, stop=True)
            else:
                # No fillers: the x@Wk pairs of this step plus the previous
                # step's transposes are the PE work that covers the tail of
                # the previous step's gate chain.
                emit_xwk(zs, x_sb, start=True, stop=False, blocks=(0, 1, 2, 3))
                emit_xwk(zs, x_sb, start=True, stop=False, blocks=(4, 5))
                state["hTa"] = hTpool.tile([128, 4 * B], BF16, name="hTa", tag="hTa")
                state["hTb"] = hTpool.tile([128, 4 * B], BF16, name="hTb", tag="hTb")
                transposes([0, 1])
                transposes([2, 3])
                emit_hwr(zs, xwk_first=True)
            lstm_rest(zs, first=(t == 0))

        # ---- decode: h@Wr first, x@Wk last, so the pred -> x_dec chain
        # hides under the recurrent matmuls ----
        for d in range(n_dec):
            zs = alloc_z()
            keep_warm(zs, 3, start=True)
            state["hTa"] = hTpool.tile([128, 4 * B], BF16, name="hTa", tag="hTa")
            state["hTb"] = hTpool.tile([128, 4 * B], BF16, name="hTb", tag="hTb")
            transposes([0, 1])
            transposes([2, 3])
            keep_warm(zs, 2)
            xd = pred_block(d)
            emit_hwr_xwk_interleaved(zs, xd)
            lstm_rest(zs, False)
        state["hTa"] = hTpool.tile([128, 4 * B], BF16, name="hTa", tag="hTa")
        state["hTb"] = hTpool.tile([128, 4 * B], BF16, name="hTb", tag="hTb")
        transposes([0, 1])
        transposes([2, 3])
        pred_block(n_out - 1)

        nc.sync.dma_start(out_ext[:], preds_sb[:])

    nc.finalize()
    _NC_CACHE[key] = nc
    return nc


def _prep_core_inputs(inputs, Wk, Wr, b, Wd, bd, n_warm, n_out):
    """Host-side reshaping/sharding. Returns list of 8 input dicts."""
    bf = lambda a: np.ascontiguousarray(a).astype(ml_dtypes.bfloat16)
    perm = np.array([g * UNITS + hh * 512 + k
                     for hh in (0, 1) for g in GATES for k in range(512)])
    Wk_aug = np.concatenate([Wk, b[None, :]], 0)[:, perm]        # [65, 4096]
    Wr_p = Wr[:, perm]                                           # [1024, 4096]
    wr_dev = bf(np.stack([Wr_p[k * 128:(k + 1) * 128] for k in range(NK)],
                         1).reshape(128, -1))
    wk_dev = bf(Wk_aug)
    wd_dev = bf(np.stack([Wd[k * 128:(k + 1) * 128] for k in range(NK)],
                         1).reshape(128, -1))
    bd_dev = np.ascontiguousarray(bd[:, None]).astype(np.float32)

    in_maps = []
    for c in range(N_CORES):
        xs = inputs[c * B:(c + 1) * B, :n_warm]                  # [64, T, F]
        xt = xs.transpose(1, 2, 0)                               # [T, F, 64]
        xt_aug = np.concatenate(
            [xt, np.ones((n_warm, 1, B), np.float32)], 1)        # [T, 65, 64]
        in_maps.append({
            "xt": bf(xt_aug), "wr": wr_dev, "wk": wk_dev,
            "wd": wd_dev, "bd": bd_dev,
        })
    return in_maps


def kernel(inputs, Wk, Wr, b, Wd, bd, out_steps):
    inputs = np.asarray(inputs, np.float32)
    Wk = np.asarray(Wk, np.float32)
    Wr = np.asarray(Wr, np.float32)
    b = np.asarray(b, np.float32)
    Wd = np.asarray(Wd, np.float32)
    bd = np.asarray(bd, np.float32)
    n_out = int(out_steps)
    n_warm = inputs.shape[1]

    nc = _build(n_warm, n_out)
    in_maps = _prep_core_inputs(inputs, Wk, Wr, b, Wd, bd, n_warm, n_out)
    res = run_bass_kernel_spmd(nc, in_maps, core_ids=list(range(N_CORES)))

    out = np.empty((B_FULL, n_out, F_DIM), np.float32)
    for c in range(N_CORES):
        o = res.results[c]["out"].reshape(F_DIM, n_out, B)       # [F, t, b]
        out[c * B:(c + 1) * B] = o.transpose(2, 1, 0)
    return out



# revision 17
# speedup vs baseline: 1.2548x; 1.0007x over previous
# Trainium2 Bass kernel for nn_AutoRegressive (LSTM warmup + autoregressive decode).
#
# Problem: B=512, T=128, F=64, UNITS=1024, OUT_STEPS=32.
#   warmup: 128 sequential LSTM steps over inputs, keep final (h, c)
#   decode: pred = h @ Wd + bd, feed pred back as x for 31 more steps
#   output: [B, 32, F]
#
# Strategy: pure 8-way data parallelism on the batch axis (64 rows/core),
# weights replicated, zero cross-core communication. Per step the dominant
# matmul z = x @ Wk + h @ Wr is computed with h^T-stationary matmuls
# (lhsT = h^T[k-chunk] [128, 64]) streaming Wr columns. Because the local
# batch is 64 (< 128 array columns), two matmuls are column-tiled at
# tile_position (0,0)/(0,64) to process the lo/hi unit-halves of each gate
# concurrently (emitted adjacently so the PE overlaps them), keeping the
# 128x128 PE array fully utilized.
# All matmul operands are bf16 (PSUM accumulates f32); the whole gate chain
# (sigmoids/tanh/c/h) is uniformly bf16 so ScalarE gets 2x accel and the DVE
# multiplies hit the 2x_1P mode. h -> h^T via 4 PE transposes per step into
# one shared PSUM bank; the freed bank gives zo a third buffer so next-step
# x@Wk-o pairs never wait the sig_o slot handoff. The h@Wr block order is
# g0,g1,f,i,o0,o1: the g tiles close first so the long c-chain
# (tanh_g -> t1 -> c -> tanh_c) overlaps the f/i/o matmuls, and only
# sig_o + the h multiply trail the last MM. ScalarE emission order matches
# the close order (tanh_g first, tanh_c/sig_o interleaved per half) to
# avoid FIFO head-blocking. Warmup: x@Wk opens the PSUM groups, h@Wr closes
# them (per-tile), no fillers -- the next step's x@Wk plus the transposes
# cover the gate-chain tail. Decode: each z tile is closed by its x@Wk pair
# right after that tile's k-loop (emit_hwr_xwk_interleaved) so gates
# pipeline during h@Wr instead of bunching after a trailing x@Wk; a few
# zero-accumulate fillers cover the transpose/pred window (HAM stays 8/8).
# pred copies run on ScalarE with bd folded in as an Identity bias. Bias b
# is folded into an augmented ones-row of x / extra row of Wk on the host.
# Measured: 1.567 ms exec on hardware (traced), rel err 6.0e-3 vs reference.
import os
import sys

sys.path.insert(0, "/opt/trn_rl_repo")

import numpy as np
import ml_dtypes

import concourse.bass as bass
import concourse.mybir as mybir
import concourse.tile as tile
from concourse import bacc
from concourse.bass_utils import run_bass_kernel_spmd
from concourse.masks import make_identity
from contextlib import ExitStack

F32, BF16 = mybir.dt.float32, mybir.dt.bfloat16
AF = mybir.ActivationFunctionType
Alu = mybir.AluOpType

B_FULL, T_FULL, F_DIM, UNITS = 512, 128, 64, 1024
N_CORES = 8
B = B_FULL // N_CORES          # 64 local batch rows
NK = UNITS // 128              # 8 k-chunks of the recurrent contraction
GATES = [1, 0, 2, 3]           # processing order f,i,g,o (orig packing i,f,g,o)

_NC_CACHE = {}


def _build(n_warm: int, n_out: int):
    """Build the per-core Bass program. n_out = number of predictions (32)."""
    key = (n_warm, n_out)
    if key in _NC_CACHE:
        return _NC_CACHE[key]

    n_dec = n_out - 1  # LSTM steps in decode phase

    nc = bacc.Bacc("TRN2", target_bir_lowering=False, debug=False,
                   num_devices=N_CORES)
    xt_ext = nc.dram_tensor("xt", [n_warm, F_DIM + 1, B], BF16,
                            kind="ExternalInput")
    wr_ext = nc.dram_tensor("wr", [128, NK * 8 * 512], BF16,
                            kind="ExternalInput")
    wk_ext = nc.dram_tensor("wk", [F_DIM + 1, 8 * 512], BF16,
                            kind="ExternalInput")
    wd_ext = nc.dram_tensor("wd", [128, NK * F_DIM], BF16,
                            kind="ExternalInput")
    bd_ext = nc.dram_tensor("bd", [F_DIM, 1], F32, kind="ExternalInput")
    out_ext = nc.dram_tensor("out", [F_DIM, n_out * B], F32,
                             kind="ExternalOutput")

    with ExitStack() as ctx:
        tc = ctx.enter_context(tile.TileContext(nc))
        wpool = ctx.enter_context(tc.tile_pool(name="w", bufs=1))
        xpool = ctx.enter_context(tc.tile_pool(name="x", bufs=3))
        hTpool = ctx.enter_context(tc.tile_pool(name="hT", bufs=2))
        hpool = ctx.enter_context(tc.tile_pool(name="h", bufs=3))
        cpool = ctx.enter_context(tc.tile_pool(name="c", bufs=3))
        gpool = ctx.enter_context(tc.tile_pool(name="g", bufs=3))
        xdpool = ctx.enter_context(tc.tile_pool(name="xd", bufs=2))
        zpool = ctx.enter_context(tc.tile_pool(name="z", bufs=1, space="PSUM"))
        zspool = ctx.enter_context(tc.tile_pool(name="zs", bufs=2, space="PSUM"))
        tpool = ctx.enter_context(tc.tile_pool(name="tp", bufs=1, space="PSUM"))

        wr_sb = wpool.tile([128, NK * 8 * 512], BF16)
        nc.sync.dma_start(wr_sb[:], wr_ext[:])
        wk_sb = wpool.tile([F_DIM + 1, 8 * 512], BF16)
        nc.sync.dma_start(wk_sb[:], wk_ext[:])
        wd_sb = wpool.tile([128, NK * F_DIM], BF16)
        nc.sync.dma_start(wd_sb[:], wd_ext[:])
        bd_sb = wpool.tile([F_DIM, 1], F32)
        nc.sync.dma_start(bd_sb[:], bd_ext[:])
        identb = wpool.tile([128, 128], BF16)
        make_identity(nc, identb[:])
        ones_sb = wpool.tile([1, B], BF16)
        nc.vector.memset(ones_sb[:], 1.0)
        zeros_sb = wpool.tile([128, 512], BF16)
        nc.vector.memset(zeros_sb[:], 0.0)
        preds_sb = wpool.tile([F_DIM, n_out * B], F32)
        xd_sb = wpool.tile([F_DIM + 1, B], BF16)
        nc.vector.memset(xd_sb[F_DIM:F_DIM + 1, :], 1.0)

        state = {"h0": None, "h1": None, "c": None, "hT": None}
        # hT column layout: transpose of h[:, j*128:(j+1)*128] yields unit
        # chunks j (cols 0:64) and j+4 (cols 64:128); store them adjacently
        # so each transpose pair needs ONE contiguous DVE copy.
        HT_POS = {}
        for j in range(4):
            HT_POS[j] = 2 * j
            HT_POS[j + 4] = 2 * j + 1

        def hT_sl(k):
            p = HT_POS[k]
            t = state["hTa"] if p < 4 else state["hTb"]
            return t[:, (p % 4) * B:(p % 4 + 1) * B]

        def transposes(js):
            """h halves (bf16, batch-major split layout) -> hT chunks (bf16).

            transpose of h[:, j*128:(j+1)*128] yields unit chunks j and j+4
            side by side. Each pair gets its own PSUM bank and its own hT
            SBUF tile so the copy of pair 0 (chunks 0,4,1,5) unblocks the
            first half of the next step's k-loop while pair 1 is still in
            flight."""
            # Each transpose pair gets its OWN PSUM bank and its own hT
            # SBUF tile: the pair-0 copy (chunks 0,4,1,5) unblocks the first
            # half of the next step's k-loop while pair 1 and its copy are
            # still in flight — real work instead of filler in the
            # HAM-activity window.
            # Both pairs share ONE PSUM bank (tag tp0): pair 1 waits for the
            # pair-0 copy, which has slack; the freed bank buys zo a third
            # buffer so next-step x@Wk-o pairs never wait the sig_o handoff.
            half = js[0] // 2
            name = "tp0" if half == 0 else "tps"
            tps = tpool.tile([128, 1024], BF16, name=name, tag="tp0")[:, 0:256]
            hT = state["hTa"] if half == 0 else state["hTb"]
            for jj, j in enumerate(js):
                h_half = state["h0"] if j < 2 else state["h1"]
                nc.tensor.transpose(tps[:, jj * 128:(jj + 1) * 128],
                                    h_half[:, (j % 2) * 128:(j % 2 + 1) * 128],
                                    identb[:])
            nc.vector.tensor_copy(hT[:], tps[:])

        def keep_warm(zs, n, start=False):
            """Filler matmuls accumulating an all-zeros K=1 product into the
            live zf tile: numerically a no-op, but real PE activity (keeps
            the HAM clock gate at 8/8 across the per-step gate-chain tail)
            that writes a consumed tile (so DCE keeps it). With start=True
            the first one opens zf's group (decode, where x@Wk comes last)."""
            zf = zs[0]
            for i in range(n):
                nc.tensor.matmul(zf[0:64, :], wr_sb[:, 0:64], zeros_sb[:],
                                 start=(start and i == 0), stop=False)

        def pred_block(d):
            """pred_d^T = Wd^T @ h + bd from current hT; returns x_dec tile."""
            # share the pair-0 transpose bank: it is released right after
            # copy-a (early), so the pred matmuls start ~0.7us sooner than
            # waiting for copy-b's slot. Safe now that the pred copies run
            # on ScalarE (the old DVE-queue release stall is gone).
            pp = tpool.tile([F_DIM, 512], F32, name="pp", tag="tp0")[:, 0:B]
            for ki, k in enumerate((0, 4, 1, 5, 2, 6, 3, 7)):
                nc.tensor.matmul(pp[:], wd_sb[:, k * F_DIM:(k + 1) * F_DIM],
                                 hT_sl(k), start=(k == 0), stop=(ki == 7))
            # Copies on ScalarE (idle here, and off the DVE queue which is
            # busy with the hT copies); bd is per-partition on pred^T so it
            # folds into the copy as an Identity bias — this also kills the
            # pathologically slow K=1 bd matmul (~630ns) from the chain.
            nc.scalar.activation(preds_sb[:, d * B:(d + 1) * B], pp[:],
                                 AF.Identity, bias=bd_sb[:])
            if d < n_out - 1:
                nc.scalar.activation(xd_sb[0:F_DIM, :], pp[:],
                                     AF.Identity, bias=bd_sb[:])
                return xd_sb
            return None

        def alloc_z():
            """Gate order f, i, g, o; g and o split into two 256-col chunks
            in separate PSUM tiles (full [128,512] banks, first 256 cols
            used — half-bank tiles would share banks and the bank-overlap
            tracker serializes reads against the bank-mate's writes)."""
            zf = zpool.tile([128, 512], F32, name="zf", tag="zf")
            zi = zpool.tile([128, 512], F32, name="zi", tag="zi")
            zg = [zspool.tile([128, 512], F32, name="zg%d" % s, tag="zg")[:, 0:256]
                  for s in (0, 1)]
            zo = [zspool.tile([128, 512], F32, name="zo%d" % s, tag="zo",
                              bufs=3)[:, 0:256]
                  for s in (0, 1)]
            return (zf, zi, zg, zo)

        def z_layout(zs):
            """Block order g0, g1, f, i, o0, o1: the g tiles close FIRST in
            h@Wr so the long c-chain (tanh_g -> t1 -> c -> tanh_c) starts
            ~3.5us before the o tiles close; after o-close only sig_o and the
            h multiply remain before the transposes."""
            zf, zi, zg, zo = zs
            return ((2, zg[0], 0, 256), (2, zg[1], 256, 256),
                    (0, zf, 0, 512), (1, zi, 0, 512),
                    (3, zo[0], 0, 256), (3, zo[1], 256, 256))

        def emit_xwk(zs, x_sb, start, stop, blocks=None):
            """x @ Wk (+b); opens the PSUM groups when start=True (warmup)
            or closes them when stop=True (decode, where it comes last).
            `blocks` selects a subset of z_layout entries (warmup emits the
            o blocks separately: they wait on the previous step's sig_o
            PSUM-slot release, and fillers must cover that window)."""
            lay = z_layout(zs)
            if blocks is not None:
                lay = [lay[b] for b in blocks]
            for blk, z, lo, n in lay:
                for half in (0, 1):
                    o = (half * 4 + blk) * 512 + lo
                    nc.tensor.matmul(z[half * 64:(half + 1) * 64, :],
                                     x_sb[:], wk_sb[:, o:o + n],
                                     start=start, stop=stop)

        def emit_hwr_xwk_interleaved(zs, x_sb):
            """Decode: h@Wr with each tile CLOSED by its x@Wk pair right
            after that tile's k-loop. Tiles then close in block order (g
            first) ~1us apart, so the gate chain pipelines during h@Wr
            exactly like warmup, instead of bunching after a trailing
            x@Wk. Groups are opened here (except zf's lo half, opened by
            keep_warm)."""
            zf = zs[0]
            for blk, z, lo, n in z_layout(zs):
                for ki, k in enumerate((0, 4, 1, 5, 2, 6, 3, 7)):
                    for half in (0, 1):
                        start = (ki == 0 and not (z is zf and half == 0))
                        o = (k * 8 + half * 4 + blk) * 512 + lo
                        nc.tensor.matmul(
                            z[half * 64:(half + 1) * 64, :],
                            hT_sl(k), wr_sb[:, o:o + n],
                            start=start, stop=False)
                for half in (0, 1):
                    o = (half * 4 + blk) * 512 + lo
                    nc.tensor.matmul(z[half * 64:(half + 1) * 64, :],
                                     x_sb[:], wk_sb[:, o:o + n],
                                     start=False, stop=True)

        def emit_hwr(zs, xwk_first):
            """h @ Wr — lo/hi column-tile pairs emitted adjacently; k-chunks
            ordered by hT readiness. When xwk_first is False (decode), this
            opens the groups (except zf, opened by keep_warm) and leaves
            them open for the trailing x@Wk."""
            zf = zs[0]
            for blk, z, lo, n in z_layout(zs):
                for ki, k in enumerate((0, 4, 1, 5, 2, 6, 3, 7)):
                    stop = xwk_first and k == NK - 1
                    for half in (0, 1):
                        # keep_warm only opened zf's lo half (partitions 0:64)
                        start = ((not xwk_first) and ki == 0
                                 and not (z is zf and half == 0))
                        o = (k * 8 + half * 4 + blk) * 512 + lo
                        nc.tensor.matmul(
                            z[half * 64:(half + 1) * 64, :],
                            hT_sl(k), wr_sb[:, o:o + n],
                            start=start, stop=stop)

        def lstm_rest(zs, first):
            """Gate math, ordered for minimal o-close -> h latency.

            ScalarE FIFO order: sig_f, sig_i, tanh_g0/1 (fire as their z
            tiles close mid-h@Wr), then tanh_c0/1 (c is ready early: it only
            needs f,i,g), then sig_o0/1 LAST (o tiles close last). DVE FIFO:
            c-chain first, the two h multiplies last. Gates are bf16 so
            ScalarE runs at 2x accel and the h multiply hits the DVE 2x_1P
            mode -- the tail after the last o matmul is just sig_o (bf16)
            + h-mult instead of a 4-deep serialized f32 chain."""
            zf, zi, zg, zo = zs
            c_prev = state["c"]
            # tanh_g first on ScalarE: the g tiles close first in h@Wr, and
            # the c-chain hangs off tanh_g -- emitting sig_f/sig_i ahead of
            # them would FIFO-block tanh_g until the f/i tiles close.
            tanh_gs = []
            for s in (0, 1):
                tg = gpool.tile([128, 256], BF16, tag="tanh_g%d" % s,
                                name="tanh_g")
                nc.scalar.activation(tg[:], zg[s][:], AF.Tanh)
                tanh_gs.append(tg)
            sig_f = None
            if not first:
                sig_f = gpool.tile([128, 512], BF16, tag="sig_f", name="sig_f")
                nc.scalar.activation(sig_f[:], zf[:], AF.Sigmoid)
            # sig_i heads the critical c-chain (zi closes last of f/i/g);
            # halving it lets t1_0 start one 256-wide ACT earlier.
            sig_i = gpool.tile([128, 512], BF16, tag="sig_i", name="sig_i")
            for s in (0, 1):
                nc.scalar.activation(sig_i[:, s * 256:(s + 1) * 256],
                                     zi[:, s * 256:(s + 1) * 256], AF.Sigmoid)
            # DVE c-chain (h multiplies are emitted after, so they don't
            # block the c ops in the DVE FIFO)
            c_new = []
            for s in (0, 1):
                sl = slice(s * 256, (s + 1) * 256)
                cs = cpool.tile([128, 256], BF16, tag="c%d" % s, name="c")
                if first:
                    nc.vector.tensor_tensor(cs[:], sig_i[:, sl], tanh_gs[s][:],
                                            Alu.mult)
                else:
                    t1 = gpool.tile([128, 256], BF16, tag="t1_%d" % s,
                                    name="t1")
                    nc.vector.tensor_tensor(t1[:], sig_i[:, sl], tanh_gs[s][:],
                                            Alu.mult)
                    nc.vector.tensor_tensor(cs[:], sig_f[:, sl], c_prev[s][:],
                                            Alu.mult)
                    nc.vector.tensor_tensor(cs[:], cs[:], t1[:], Alu.add)
                c_new.append(cs)
            # Tail, interleaved per half: tanh_c0, sig_o0 (h0's inputs) fire
            # before tanh_c1/sig_o1 on the Scalar FIFO, so h0 -- and with it
            # transpose pair 0 and the next step's first k-chunks -- is ready
            # two ACTs earlier. tanh_c before sig_o within each half because
            # c is ready well before the o tiles close.
            tanh_cs, sig_os = [], []
            for s in (0, 1):
                tc_s = gpool.tile([128, 256], BF16, tag="tanh_c%d" % s,
                                  name="tanh_c")
                nc.scalar.activation(tc_s[:], c_new[s][:], AF.Tanh)
                tanh_cs.append(tc_s)
                so = gpool.tile([128, 256], BF16, tag="sig_o%d" % s,
                                name="sig_o")
                nc.scalar.activation(so[:], zo[s][:], AF.Sigmoid)
                sig_os.append(so)
            h_new = []
            for s in (0, 1):
                hs = hpool.tile([128, 256], BF16, tag="h%d" % s, name="h")
                nc.vector.tensor_tensor(hs[:], sig_os[s][:], tanh_cs[s][:],
                                        Alu.mult)
                h_new.append(hs)
            state["h0"], state["h1"], state["c"] = h_new[0], h_new[1], c_new

        # ---- warmup ----
        for t in range(n_warm):
            x_sb = xpool.tile([F_DIM + 1, B], BF16)
            nc.sync.dma_start(x_sb[:], xt_ext[t])
            zs = alloc_z()
            if t == 0:
                emit_xwk(zs, x_sb, start=True, stop=True)
            else:
                # No fillers: the x@Wk pairs of this step plus the previous
                # step's transposes are the PE work that covers the tail of
                # the previous step's gate chain.
                emit_xwk(zs, x_sb, start=True, stop=False, blocks=(0, 1, 2, 3))
                emit_xwk(zs, x_sb, start=True, stop=False, blocks=(4, 5))
                state["hTa"] = hTpool.tile([128, 4 * B], BF16, name="hTa", tag="hTa")
                state["hTb"] = hTpool.tile([128, 4 * B], BF16, name="hTb", tag="hTb")
                transposes([0, 1])
                transposes([2, 3])
                emit_hwr(zs, xwk_first=True)
            lstm_rest(zs, first=(t == 0))

        # ---- decode: h@Wr first, x@Wk last, so the pred -> x_dec chain
        # hides under the recurrent matmuls ----
        for d in range(n_dec):
            zs = alloc_z()
            keep_warm(zs, 3, start=True)
            state["hTa"] = hTpool.tile([128, 4 * B], BF16, name="hTa", tag="hTa")
            state["hTb"] = hTpool.tile([128, 4 * B], BF16, name="hTb", tag="hTb")
            transposes([0, 1])
            transposes([2, 3])
            keep_warm(zs, 2)
            xd = pred_block(d)
            emit_hwr_xwk_interleaved(zs, xd)
            lstm_rest(zs, False)
        state["hTa"] = hTpool.tile([128, 4 * B], BF16, name="hTa", tag="hTa")
        state["hTb"] = hTpool.tile([128, 4 * B], BF16, name="hTb", tag="hTb")
        transposes([0, 1])
        transposes([2, 3])
        pred_block(n_out - 1)

        nc.sync.dma_start(out_ext[:], preds_sb[:])

    nc.finalize()
    _NC_CACHE[key] = nc
    return nc


def _prep_core_inputs(inputs, Wk, Wr, b, Wd, bd, n_warm, n_out):
    """Host-side reshaping/sharding. Returns list of 8 input dicts."""
    bf = lambda a: np.ascontiguousarray(a).astype(ml_dtypes.bfloat16)
    perm = np.array([g * UNITS + hh * 512 + k
                     for hh in (0, 1) for g in GATES for k in range(512)])
    Wk_aug = np.concatenate([Wk, b[None, :]], 0)[:, perm]        # [65, 4096]
    Wr_p = Wr[:, perm]                                           # [1024, 4096]
    wr_dev = bf(np.stack([Wr_p[k * 128:(k + 1) * 128] for k in range(NK)],
                         1).reshape(128, -1))
    wk_dev = bf(Wk_aug)
    wd_dev = bf(np.stack([Wd[k * 128:(k + 1) * 128] for k in range(NK)],
                         1).reshape(128, -1))
    bd_dev = np.ascontiguousarray(bd[:, None]).astype(np.float32)

    in_maps = []
    for c in range(N_CORES):
        xs = inputs[c * B:(c + 1) * B, :n_warm]                  # [64, T, F]
        xt = xs.transpose(1, 2, 0)                               # [T, F, 64]
        xt_aug = np.concatenate(
            [xt, np.ones((n_warm, 1, B), np.float32)], 1)        # [T, 65, 64]
        in_maps.append({
            "xt": bf(xt_aug), "wr": wr_dev, "wk": wk_dev,
            "wd": wd_dev, "bd": bd_dev,
        })
    return in_maps


def kernel(inputs, Wk, Wr, b, Wd, bd, out_steps):
    inputs = np.asarray(inputs, np.float32)
    Wk = np.asarray(Wk, np.float32)
    Wr = np.asarray(Wr, np.float32)
    b = np.asarray(b, np.float32)
    Wd = np.asarray(Wd, np.float32)
    bd = np.asarray(bd, np.float32)
    n_out = int(out_steps)
    n_warm = inputs.shape[1]

    nc = _build(n_warm, n_out)
    in_maps = _prep_core_inputs(inputs, Wk, Wr, b, Wd, bd, n_warm, n_out)
    res = run_bass_kernel_spmd(nc, in_maps, core_ids=list(range(N_CORES)))

    out = np.empty((B_FULL, n_out, F_DIM), np.float32)
    for c in range(N_CORES):
        o = res.results[c]["out"].reshape(F_DIM, n_out, B)       # [F, t, b]
        out[c * B:(c + 1) * B] = o.transpose(2, 1, 0)
    return out



# revision 19
# speedup vs baseline: 1.2568x; 1.0016x over previous
# Trainium2 Bass kernel for nn_AutoRegressive (LSTM warmup + autoregressive decode).
#
# Problem: B=512, T=128, F=64, UNITS=1024, OUT_STEPS=32.
#   warmup: 128 sequential LSTM steps over inputs, keep final (h, c)
#   decode: pred = h @ Wd + bd, feed pred back as x for 31 more steps
#   output: [B, 32, F]
#
# Strategy: pure 8-way data parallelism on the batch axis (64 rows/core),
# weights replicated, zero cross-core communication. Per step the dominant
# matmul z = x @ Wk + h @ Wr is computed with h^T-stationary matmuls
# (lhsT = h^T[k-chunk] [128, 64]) streaming Wr columns. Because the local
# batch is 64 (< 128 array columns), two matmuls are column-tiled at
# tile_position (0,0)/(0,64) to process the lo/hi unit-halves of each gate
# concurrently (emitted adjacently so the PE overlaps them), keeping the
# 128x128 PE array fully utilized.
# All matmul operands are bf16 (PSUM accumulates f32); the whole gate chain
# (sigmoids/tanh/c/h) is uniformly bf16 so ScalarE gets 2x accel and the DVE
# multiplies hit the 2x_1P mode. h -> h^T via 4 PE transposes per step into
# one shared PSUM bank; the freed bank gives zo a third buffer so next-step
# x@Wk-o pairs never wait the sig_o slot handoff. The h@Wr block order is
# g0,g1,f,i,o0,o1: the g tiles close first so the long c-chain
# (tanh_g -> t1 -> c -> tanh_c) overlaps the f/i/o matmuls, and only
# sig_o + the h multiply trail the last MM. ScalarE emission order matches
# the close order (tanh_g first, tanh_c/sig_o interleaved per half) to
# avoid FIFO head-blocking. Warmup: x@Wk opens the PSUM groups, h@Wr closes
# them (per-tile), no fillers -- the next step's x@Wk plus the transposes
# cover the gate-chain tail. Decode: each z tile is closed by its x@Wk pair
# right after that tile's k-loop (emit_hwr_xwk_interleaved) so gates
# pipeline during h@Wr instead of bunching after a trailing x@Wk; a few
# zero-accumulate fillers cover the transpose/pred window (HAM stays 8/8).
# pred copies run on ScalarE with bd folded in as an Identity bias. Bias b
# is folded into an augmented ones-row of x / extra row of Wk on the host.
# Measured: 1.567 ms exec on hardware (traced), rel err 6.0e-3 vs reference.
import os
import sys

sys.path.insert(0, "/opt/trn_rl_repo")

import numpy as np
import ml_dtypes

import concourse.bass as bass
import concourse.mybir as mybir
import concourse.tile as tile
from concourse import bacc
from concourse.bass_utils import run_bass_kernel_spmd
from concourse.masks import make_identity
from contextlib import ExitStack

F32, BF16 = mybir.dt.float32, mybir.dt.bfloat16
AF = mybir.ActivationFunctionType
Alu = mybir.AluOpType

B_FULL, T_FULL, F_DIM, UNITS = 512, 128, 64, 1024
N_CORES = 8
B = B_FULL // N_CORES          # 64 local batch rows
NK = UNITS // 128              # 8 k-chunks of the recurrent contraction
GATES = [1, 0, 2, 3]           # processing order f,i,g,o (orig packing i,f,g,o)

_NC_CACHE = {}


def _build(n_warm: int, n_out: int):
    """Build the per-core Bass program. n_out = number of predictions (32)."""
    key = (n_warm, n_out)
    if key in _NC_CACHE:
        return _NC_CACHE[key]

    n_dec = n_out - 1  # LSTM steps in decode phase

    nc = bacc.Bacc("TRN2", target_bir_lowering=False, debug=False,
                   num_devices=N_CORES)
    xt_ext = nc.dram_tensor("xt", [n_warm, F_DIM + 1, B], BF16,
                            kind="ExternalInput")
    wr_ext = nc.dram_tensor("wr", [128, NK * 8 * 512], BF16,
                            kind="ExternalInput")
    wk_ext = nc.dram_tensor("wk", [F_DIM + 1, 8 * 512], BF16,
                            kind="ExternalInput")
    wd_ext = nc.dram_tensor("wd", [128, NK * F_DIM], BF16,
                            kind="ExternalInput")
    bd_ext = nc.dram_tensor("bd", [F_DIM, 1], F32, kind="ExternalInput")
    out_ext = nc.dram_tensor("out", [F_DIM, n_out * B], F32,
                             kind="ExternalOutput")

    with ExitStack() as ctx:
        tc = ctx.enter_context(tile.TileContext(nc))
        wpool = ctx.enter_context(tc.tile_pool(name="w", bufs=1))
        xpool = ctx.enter_context(tc.tile_pool(name="x", bufs=3))
        hTpool = ctx.enter_context(tc.tile_pool(name="hT", bufs=2))
        hpool = ctx.enter_context(tc.tile_pool(name="h", bufs=3))
        cpool = ctx.enter_context(tc.tile_pool(name="c", bufs=3))
        gpool = ctx.enter_context(tc.tile_pool(name="g", bufs=3))
        xdpool = ctx.enter_context(tc.tile_pool(name="xd", bufs=2))
        zpool = ctx.enter_context(tc.tile_pool(name="z", bufs=1, space="PSUM"))
        zspool = ctx.enter_context(tc.tile_pool(name="zs", bufs=2, space="PSUM"))
        tpool = ctx.enter_context(tc.tile_pool(name="tp", bufs=1, space="PSUM"))

        wr_sb = wpool.tile([128, NK * 8 * 512], BF16)
        nc.sync.dma_start(wr_sb[:], wr_ext[:])
        wk_sb = wpool.tile([F_DIM + 1, 8 * 512], BF16)
        nc.sync.dma_start(wk_sb[:], wk_ext[:])
        wd_sb = wpool.tile([128, NK * F_DIM], BF16)
        nc.sync.dma_start(wd_sb[:], wd_ext[:])
        bd_sb = wpool.tile([F_DIM, 1], F32)
        nc.sync.dma_start(bd_sb[:], bd_ext[:])
        identb = wpool.tile([128, 128], BF16)
        make_identity(nc, identb[:])
        ones_sb = wpool.tile([1, B], BF16)
        nc.vector.memset(ones_sb[:], 1.0)
        zeros_sb = wpool.tile([128, 512], BF16)
        nc.vector.memset(zeros_sb[:], 0.0)
        preds_sb = wpool.tile([F_DIM, n_out * B], F32)
        xd_sb = wpool.tile([F_DIM + 1, B], BF16)
        nc.vector.memset(xd_sb[F_DIM:F_DIM + 1, :], 1.0)

        state = {"h0": None, "h1": None, "c": None, "hT": None}
        # hT column layout: transpose of h[:, j*128:(j+1)*128] yields unit
        # chunks j (cols 0:64) and j+4 (cols 64:128); store them adjacently
        # so each transpose pair needs ONE contiguous DVE copy.
        HT_POS = {}
        for j in range(4):
            HT_POS[j] = 2 * j
            HT_POS[j + 4] = 2 * j + 1

        def hT_sl(k):
            p = HT_POS[k]
            t = state["hTa"] if p < 4 else state["hTb"]
            return t[:, (p % 4) * B:(p % 4 + 1) * B]

        def transposes(js):
            """h halves (bf16, batch-major split layout) -> hT chunks (bf16).

            transpose of h[:, j*128:(j+1)*128] yields unit chunks j and j+4
            side by side. Each pair gets its own PSUM bank and its own hT
            SBUF tile so the copy of pair 0 (chunks 0,4,1,5) unblocks the
            first half of the next step's k-loop while pair 1 is still in
            flight."""
            # Each transpose pair gets its OWN PSUM bank and its own hT
            # SBUF tile: the pair-0 copy (chunks 0,4,1,5) unblocks the first
            # half of the next step's k-loop while pair 1 and its copy are
            # still in flight — real work instead of filler in the
            # HAM-activity window.
            # Both pairs live in ONE PSUM tile (separate column ranges) so
            # the subtile dependency tracker lets pair 1's transposes run
            # while the pair-0 copy is still reading columns 0:256. One bank
            # total: the freed bank buys zo a third buffer so next-step
            # x@Wk-o pairs never wait the sig_o handoff.
            half = js[0] // 2
            if half == 0:
                state["tp"] = tpool.tile([128, 1024], BF16, name="tp",
                                         tag="tp0")
            tps = state["tp"][:, half * 256:(half + 1) * 256]
            hT = state["hTa"] if half == 0 else state["hTb"]
            for jj, j in enumerate(js):
                h_half = state["h0"] if j < 2 else state["h1"]
                nc.tensor.transpose(tps[:, jj * 128:(jj + 1) * 128],
                                    h_half[:, (j % 2) * 128:(j % 2 + 1) * 128],
                                    identb[:])
            nc.vector.tensor_copy(hT[:], tps[:])

        def keep_warm(zs, n, start=False):
            """Filler matmuls accumulating an all-zeros K=1 product into the
            live zf tile: numerically a no-op, but real PE activity (keeps
            the HAM clock gate at 8/8 across the per-step gate-chain tail)
            that writes a consumed tile (so DCE keeps it). With start=True
            the first one opens zf's group (decode, where x@Wk comes last)."""
            zf = zs[0]
            for i in range(n):
                nc.tensor.matmul(zf[0:64, :], wr_sb[:, 0:64], zeros_sb[:],
                                 start=(start and i == 0), stop=False)

        def pred_block(d):
            """pred_d^T = Wd^T @ h + bd from current hT; returns x_dec tile."""
            # share the pair-0 transpose bank: it is released right after
            # copy-a (early), so the pred matmuls start ~0.7us sooner than
            # waiting for copy-b's slot. Safe now that the pred copies run
            # on ScalarE (the old DVE-queue release stall is gone).
            pp = tpool.tile([F_DIM, 512], F32, name="pp", tag="tp0")[:, 0:B]
            for ki, k in enumerate((0, 4, 1, 5, 2, 6, 3, 7)):
                nc.tensor.matmul(pp[:], wd_sb[:, k * F_DIM:(k + 1) * F_DIM],
                                 hT_sl(k), start=(k == 0), stop=(ki == 7))
            # Copies on ScalarE (idle here, and off the DVE queue which is
            # busy with the hT copies); bd is per-partition on pred^T so it
            # folds into the copy as an Identity bias — this also kills the
            # pathologically slow K=1 bd matmul (~630ns) from the chain.
            nc.scalar.activation(preds_sb[:, d * B:(d + 1) * B], pp[:],
                                 AF.Identity, bias=bd_sb[:])
            if d < n_out - 1:
                nc.scalar.activation(xd_sb[0:F_DIM, :], pp[:],
                                     AF.Identity, bias=bd_sb[:])
                return xd_sb
            return None

        def alloc_z():
            """Gate order f, i, g, o; g and o split into two 256-col chunks
            in separate PSUM tiles (full [128,512] banks, first 256 cols
            used — half-bank tiles would share banks and the bank-overlap
            tracker serializes reads against the bank-mate's writes)."""
            zf = zpool.tile([128, 512], F32, name="zf", tag="zf")
            zi = zpool.tile([128, 512], F32, name="zi", tag="zi")
            zg = [zspool.tile([128, 512], F32, name="zg%d" % s, tag="zg")[:, 0:256]
                  for s in (0, 1)]
            zo = [zspool.tile([128, 512], F32, name="zo%d" % s, tag="zo",
                              bufs=3)[:, 0:256]
                  for s in (0, 1)]
            return (zf, zi, zg, zo)

        def z_layout(zs):
            """Block order g0, g1, f, i, o0, o1: the g tiles close FIRST in
            h@Wr so the long c-chain (tanh_g -> t1 -> c -> tanh_c) starts
            ~3.5us before the o tiles close; after o-close only sig_o and the
            h multiply remain before the transposes."""
            zf, zi, zg, zo = zs
            return ((2, zg[0], 0, 256), (2, zg[1], 256, 256),
                    (0, zf, 0, 512), (1, zi, 0, 512),
                    (3, zo[0], 0, 256), (3, zo[1], 256, 256))

        def emit_xwk(zs, x_sb, start, stop, blocks=None):
            """x @ Wk (+b); opens the PSUM groups when start=True (warmup)
            or closes them when stop=True (decode, where it comes last).
            `blocks` selects a subset of z_layout entries (warmup emits the
            o blocks separately: they wait on the previous step's sig_o
            PSUM-slot release, and fillers must cover that window)."""
            lay = z_layout(zs)
            if blocks is not None:
                lay = [lay[b] for b in blocks]
            for blk, z, lo, n in lay:
                for half in (0, 1):
                    o = (half * 4 + blk) * 512 + lo
                    nc.tensor.matmul(z[half * 64:(half + 1) * 64, :],
                                     x_sb[:], wk_sb[:, o:o + n],
                                     start=start, stop=stop)

        def emit_hwr_xwk_interleaved(zs, x_sb):
            """Decode: h@Wr with each tile CLOSED by its x@Wk pair right
            after that tile's k-loop. Tiles then close in block order (g
            first) ~1us apart, so the gate chain pipelines during h@Wr
            exactly like warmup, instead of bunching after a trailing
            x@Wk. Groups are opened here (except zf's lo half, opened by
            keep_warm)."""
            zf = zs[0]
            for blk, z, lo, n in z_layout(zs):
                for ki, k in enumerate((0, 4, 1, 5, 2, 6, 3, 7)):
                    for half in (0, 1):
                        start = (ki == 0 and not (z is zf and half == 0))
                        o = (k * 8 + half * 4 + blk) * 512 + lo
                        nc.tensor.matmul(
                            z[half * 64:(half + 1) * 64, :],
                            hT_sl(k), wr_sb[:, o:o + n],
                            start=start, stop=False)
                for half in (0, 1):
                    o = (half * 4 + blk) * 512 + lo
                    nc.tensor.matmul(z[half * 64:(half + 1) * 64, :],
                                     x_sb[:], wk_sb[:, o:o + n],
                                     start=False, stop=True)

        def emit_hwr(zs, xwk_first):
            """h @ Wr — lo/hi column-tile pairs emitted adjacently; k-chunks
            ordered by hT readiness. When xwk_first is False (decode), this
            opens the groups (except zf, opened by keep_warm) and leaves
            them open for the trailing x@Wk."""
            zf = zs[0]
            for blk, z, lo, n in z_layout(zs):
                for ki, k in enumerate((0, 4, 1, 5, 2, 6, 3, 7)):
                    stop = xwk_first and k == NK - 1
                    for half in (0, 1):
                        # keep_warm only opened zf's lo half (partitions 0:64)
                        start = ((not xwk_first) and ki == 0
                                 and not (z is zf and half == 0))
                        o = (k * 8 + half * 4 + blk) * 512 + lo
                        nc.tensor.matmul(
                            z[half * 64:(half + 1) * 64, :],
                            hT_sl(k), wr_sb[:, o:o + n],
                            start=start, stop=stop)

        def lstm_rest(zs, first):
            """Gate math, ordered for minimal o-close -> h latency.

            ScalarE FIFO order: sig_f, sig_i, tanh_g0/1 (fire as their z
            tiles close mid-h@Wr), then tanh_c0/1 (c is ready early: it only
            needs f,i,g), then sig_o0/1 LAST (o tiles close last). DVE FIFO:
            c-chain first, the two h multiplies last. Gates are bf16 so
            ScalarE runs at 2x accel and the h multiply hits the DVE 2x_1P
            mode -- the tail after the last o matmul is just sig_o (bf16)
            + h-mult instead of a 4-deep serialized f32 chain."""
            zf, zi, zg, zo = zs
            c_prev = state["c"]
            # tanh_g first on ScalarE: the g tiles close first in h@Wr, and
            # the c-chain hangs off tanh_g -- emitting sig_f/sig_i ahead of
            # them would FIFO-block tanh_g until the f/i tiles close.
            tanh_gs = []
            for s in (0, 1):
                tg = gpool.tile([128, 256], BF16, tag="tanh_g%d" % s,
                                name="tanh_g")
                nc.scalar.activation(tg[:], zg[s][:], AF.Tanh)
                tanh_gs.append(tg)
            sig_f = None
            if not first:
                sig_f = gpool.tile([128, 512], BF16, tag="sig_f", name="sig_f")
                nc.scalar.activation(sig_f[:], zf[:], AF.Sigmoid)
            # sig_i heads the critical c-chain (zi closes last of f/i/g);
            # halving it lets t1_0 start one 256-wide ACT earlier.
            sig_i = gpool.tile([128, 512], BF16, tag="sig_i", name="sig_i")
            for s in (0, 1):
                nc.scalar.activation(sig_i[:, s * 256:(s + 1) * 256],
                                     zi[:, s * 256:(s + 1) * 256], AF.Sigmoid)
            # DVE c-chain (h multiplies are emitted after, so they don't
            # block the c ops in the DVE FIFO)
            c_new = []
            for s in (0, 1):
                sl = slice(s * 256, (s + 1) * 256)
                cs = cpool.tile([128, 256], BF16, tag="c%d" % s, name="c")
                if first:
                    nc.vector.tensor_tensor(cs[:], sig_i[:, sl], tanh_gs[s][:],
                                            Alu.mult)
                else:
                    t1 = gpool.tile([128, 256], BF16, tag="t1_%d" % s,
                                    name="t1")
                    nc.vector.tensor_tensor(t1[:], sig_i[:, sl], tanh_gs[s][:],
                                            Alu.mult)
                    nc.vector.tensor_tensor(cs[:], sig_f[:, sl], c_prev[s][:],
                                            Alu.mult)
                    nc.vector.tensor_tensor(cs[:], cs[:], t1[:], Alu.add)
                c_new.append(cs)
            # Tail, interleaved per half with sig_o FIRST: zo0 closes at
            # o0-close, EARLIER than c0 emerges from the DVE c-chain, so
            # sig_o0 heads the FIFO and tanh_c0 lands right as c0 is ready.
            # h0's inputs (sig_o0, tanh_c0) complete two ACTs before h1's,
            # pulling transpose pair 0 and the next step's first k-chunks
            # forward.
            tanh_cs, sig_os = [], []
            for s in (0, 1):
                so = gpool.tile([128, 256], BF16, tag="sig_o%d" % s,
                                name="sig_o")
                nc.scalar.activation(so[:], zo[s][:], AF.Sigmoid)
                sig_os.append(so)
                tc_s = gpool.tile([128, 256], BF16, tag="tanh_c%d" % s,
                                  name="tanh_c")
                nc.scalar.activation(tc_s[:], c_new[s][:], AF.Tanh)
                tanh_cs.append(tc_s)
            h_new = []
            for s in (0, 1):
                hs = hpool.tile([128, 256], BF16, tag="h%d" % s, name="h")
                nc.vector.tensor_tensor(hs[:], sig_os[s][:], tanh_cs[s][:],
                                        Alu.mult)
                h_new.append(hs)
            state["h0"], state["h1"], state["c"] = h_new[0], h_new[1], c_new

        # ---- warmup ----
        for t in range(n_warm):
            x_sb = xpool.tile([F_DIM + 1, B], BF16)
            nc.sync.dma_start(x_sb[:], xt_ext[t])
            zs = alloc_z()
            if t == 0:
                emit_xwk(zs, x_sb, start=True, stop=True)
            else:
                # No fillers: the x@Wk pairs of this step plus the previous
                # step's transposes are the PE work that covers the tail of
                # the previous step's gate chain.
                emit_xwk(zs, x_sb, start=True, stop=False, blocks=(0, 1, 2, 3))
                emit_xwk(zs, x_sb, start=True, stop=False, blocks=(4, 5))
                state["hTa"] = hTpool.tile([128, 4 * B], BF16, name="hTa", tag="hTa")
                state["hTb"] = hTpool.tile([128, 4 * B], BF16, name="hTb", tag="hTb")
                transposes([0, 1])
                transposes([2, 3])
                emit_hwr(zs, xwk_first=True)
            lstm_rest(zs, first=(t == 0))

        # ---- decode: h@Wr first, x@Wk last, so the pred -> x_dec chain
        # hides under the recurrent matmuls ----
        for d in range(n_dec):
            zs = alloc_z()
            keep_warm(zs, 3, start=True)
            state["hTa"] = hTpool.tile([128, 4 * B], BF16, name="hTa", tag="hTa")
            state["hTb"] = hTpool.tile([128, 4 * B], BF16, name="hTb", tag="hTb")
            transposes([0, 1])
            transposes([2, 3])
            keep_warm(zs, 2)
            xd = pred_block(d)
            emit_hwr_xwk_interleaved(zs, xd)
            lstm_rest(zs, False)
        state["hTa"] = hTpool.tile([128, 4 * B], BF16, name="hTa", tag="hTa")
        state["hTb"] = hTpool.tile([128, 4 * B], BF16, name="hTb", tag="hTb")
        transposes([0, 1])
        transposes([2, 3])
        pred_block(n_out - 1)

        nc.sync.dma_start(out_ext[:], preds_sb[:])

    nc.finalize()
    _NC_CACHE[key] = nc
    return nc


def _prep_core_inputs(inputs, Wk, Wr, b, Wd, bd, n_warm, n_out):
    """Host-side reshaping/sharding. Returns list of 8 input dicts."""
    bf = lambda a: np.ascontiguousarray(a).astype(ml_dtypes.bfloat16)
    perm = np.array([g * UNITS + hh * 512 + k
                     for hh in (0, 1) for g in GATES for k in range(512)])
    Wk_aug = np.concatenate([Wk, b[None, :]], 0)[:, perm]        # [65, 4096]
    Wr_p = Wr[:, perm]                                           # [1024, 4096]
    wr_dev = bf(np.stack([Wr_p[k * 128:(k + 1) * 128] for k in range(NK)],
                         1).reshape(128, -1))
    wk_dev = bf(Wk_aug)
    wd_dev = bf(np.stack([Wd[k * 128:(k + 1) * 128] for k in range(NK)],
                         1).reshape(128, -1))
    bd_dev = np.ascontiguousarray(bd[:, None]).astype(np.float32)

    in_maps = []
    for c in range(N_CORES):
        xs = inputs[c * B:(c + 1) * B, :n_warm]                  # [64, T, F]
        xt = xs.transpose(1, 2, 0)                               # [T, F, 64]
        xt_aug = np.concatenate(
            [xt, np.ones((n_warm, 1, B), np.float32)], 1)        # [T, 65, 64]
        in_maps.append({
            "xt": bf(xt_aug), "wr": wr_dev, "wk": wk_dev,
            "wd": wd_dev, "bd": bd_dev,
        })
    return in_maps


def kernel(inputs, Wk, Wr, b, Wd, bd, out_steps):
    inputs = np.asarray(inputs, np.float32)
    Wk = np.asarray(Wk, np.float32)
    Wr = np.asarray(Wr, np.float32)
    b = np.asarray(b, np.float32)
    Wd = np.asarray(Wd, np.float32)
    bd = np.asarray(bd, np.float32)
    n_out = int(out_steps)
    n_warm = inputs.shape[1]

    nc = _build(n_warm, n_out)
    in_maps = _prep_core_inputs(inputs, Wk, Wr, b, Wd, bd, n_warm, n_out)
    res = run_bass_kernel_spmd(nc, in_maps, core_ids=list(range(N_CORES)))

    out = np.empty((B_FULL, n_out, F_DIM), np.float32)
    for c in range(N_CORES):
        o = res.results[c]["out"].reshape(F_DIM, n_out, B)       # [F, t, b]
        out[c * B:(c + 1) * B] = o.transpose(2, 1, 0)
    return out

